# revision 55
# baseline (speedup 1.0000x reference)
"""Trainium2 Bass kernel for causal multi-head attention + output projection.

Problem: B=2, S=2048, D=1024, H=16 heads of HD=64; fp32; causal softmax
scaled by D**-0.5; output projection with bias.

Sharding: 2 heads per core (tensor parallel on heads) for QKV + attention,
then an on-device AllToAll reshards from head-split to sequence-split and
each core computes its 512 rows of the output projection locally.

v3 schedule notes (vs v2 baseline):
 - single unified 80-tick pipeline: per tick both heads' score matmuls run
   concurrently as 2-way row-tiled K=64 matmuls (tile_position via base
   partitions 0/64), one exp per tick covers both heads.
 - AV uses fp8 DoubleRow over key-tile pairs (2 key tiles per matmul pass),
   with the softmax-denominator ones-row riding as output row 64.
 - all of x is fp8 (no bf16 copy): Q,K,V projections are all DoubleRow.
 - the two heads' A2A payloads are merged into one collective per batch.
 - QKV projection / V-transpose / phase-D work runs as paced fillers inside
   the attention tick stream, so the scalar-engine exp is never exposed.
"""

import sys

sys.path.insert(0, "/opt/trn_rl_repo")

import numpy as np

import concourse.bacc as bacc
import concourse.mybir as mybir
import concourse.tile as tile

B, D, H, HD = 2, 1024, 16, 64
NCORES = 8
SCALE = float(D) ** -0.5
QK_PRESCALE = 8.0  # Wq,Wk,Wv scaled by 8 before fp8 cast (avoids subnormals)
F32 = mybir.dt.float32
F32R = mybir.dt.float32r
BF16 = mybir.dt.bfloat16
F8E4 = mybir.dt.float8e4
DoubleRow = mybir.MatmulPerfMode.DoubleRow
Exp = mybir.ActivationFunctionType.Exp


def build(S=2048, dump=False):
    KD = D // 128          # contraction tiles for the projections (8)
    KT = KD // 2           # fp8 DoubleRow contraction pair-tiles (4)
    NT = S // 128          # key tiles
    SQ = 512               # query-chunk width
    NCH = S // SQ          # query chunks per (batch, head)
    HSL = S // NCORES      # rows of output owned per core per batch
    NST = HSL // 128       # 128-row tiles per core per batch
    NNN = D // 512         # 512-col output chunks

    nc = bacc.Bacc("TRN2", target_bir_lowering=False, debug=False)
    # fp8 x arranged for DoubleRow: [b, tt, p, kt, s] with d = 256*tt + 128*kt + p
    xT8 = nc.dram_tensor("xT8", [B, KT, 128, 2, S], F8E4, kind="ExternalInput")
    # q,k weights (x QK_PRESCALE) for DoubleRow: [p, w, tt, kt, m]
    Wqk8 = nc.dram_tensor("Wqk8", [128, 2, KT, 2, 128], F8E4, kind="ExternalInput")
    # bf16 x (for the v projection; fp8 x is too lossy for values)
    xT = nc.dram_tensor("xT", [B, D, S], BF16, kind="ExternalInput")
    Wv = nc.dram_tensor("Wv", [128, KD, 128], BF16, kind="ExternalInput")
    WpT = nc.dram_tensor("WpT", [128, KD, D], BF16, kind="ExternalInput")
    bp = nc.dram_tensor("bp", [1, D], BF16, kind="ExternalInput")
    # causal mask for the diagonal 128-block, replicated for the 2 heads
    mask2 = nc.dram_tensor("mask2", [128, 2, 128], F8E4, kind="ExternalInput")
    mask2b = nc.dram_tensor("mask2b", [128, 2, 128], BF16, kind="ExternalInput")
    idin = nc.dram_tensor("idin", [128, 128], BF16, kind="ExternalInput")
    sel = nc.dram_tensor("sel", [40, KD, 128], F32, kind="ExternalInput")
    # y rows: [0:HSL] = batch0 s-slice, [HSL:2*HSL] = batch1 s-slice
    y = nc.dram_tensor("y", [B * HSL, D], F32, kind="ExternalOutput")

    with tile.TileContext(nc) as tc:
        ctx_pools = [
            tc.tile_pool(name="persist", bufs=1),
            tc.tile_pool(name="dram", bufs=1, space="DRAM"),
            tc.tile_pool(name="wq", bufs=1),
            tc.tile_pool(name="xp", bufs=2),
            tc.tile_pool(name="qk", bufs=2),
            tc.tile_pool(name="vp", bufs=2),
            tc.tile_pool(name="at", bufs=4),
            tc.tile_pool(name="at16", bufs=2),
            tc.tile_pool(name="xsl", bufs=2),
            tc.tile_pool(name="prj", bufs=2),
            tc.tile_pool(name="yo", bufs=2),
            tc.tile_pool(name="ps_mix", bufs=2, space="PSUM"),
            tc.tile_pool(name="ps_sc", bufs=2, space="PSUM"),
            tc.tile_pool(name="ps_oT", bufs=2, space="PSUM"),
        ]
        import contextlib

        with contextlib.ExitStack() as stk:
            (
                persist, dram, wpool, xpool, qkpool, vppool, atpool,
                at16pool, xslpool,
                prjpool, ypool, ps_mix, ps_sc, ps_oT,
            ) = [stk.enter_context(p) for p in ctx_pools]

            # ---- critical-path first: small constants, weights, batch-0 x ----
            ident = persist.tile([128, 128], BF16)
            nc.sync.dma_start(out=ident, in_=idin[:, :])
            mask_sb = persist.tile([128, 2, 128], F8E4)
            nc.sync.dma_start(out=mask_sb, in_=mask2[:, :, :])
            mask_sb16 = persist.tile([128, 2, 128], BF16)
            nc.sync.dma_start(out=mask_sb16, in_=mask2b[:, :, :])
            wqk8_sb = wpool.tile([128, 2, KT, 2, 128], F8E4, tag="wqk8")
            nc.sync.dma_start(out=wqk8_sb, in_=Wqk8[:, :, :, :, :])
            wv_sb = wpool.tile([128, KD, 128], BF16, tag="wv")
            nc.sync.dma_start(out=wv_sb, in_=Wv[:, :, :])

            _warm_n = [0]

            def warm(k):
                # back-to-back thin matmuls: keeps the PE HAM activity window
                # busy across DMA / collective waits; 1-col stationary makes
                # the LDWEIGHTS cost negligible
                _warm_n[0] += 1
                wps = ps_mix.tile(
                    [128, SQ], F32, tag="mix", name=f"warm_{_warm_n[0]}"
                )
                for _ in range(k):
                    nc.tensor.matmul(
                        wps[0:1, 0:128], ident[:, 0:1], ident, start=True, stop=True
                    )

            warm(26)  # bridge until the first x slab lands

            x8_sb, x_sb = {}, {}

            HS = S // 2  # half-sequence slab width (bigger DMA pieces)
            CPH = HS // SQ  # query chunks per half-slab

            def load_x8slab(b, hn):
                # fp8 x half-slab hn for batch b (1KB contiguous pieces)
                if b not in x8_sb:
                    x8_sb[b] = [
                        xpool.tile(
                            [128, 2, S], F8E4, tag=f"x8{tt}", name=f"x8_{b}_{tt}"
                        )
                        for tt in range(KT)
                    ]
                # scalar HWDGE queue: runs parallel to the sync queue that
                # carries the (2x bigger) bf16 stream
                for tt in range(KT):
                    nc.scalar.dma_start(
                        out=x8_sb[b][tt][:, :, HS * hn : HS * (hn + 1)],
                        in_=xT8[b, tt, :, :, HS * hn : HS * (hn + 1)],
                    )

            def load_xslab(b, hn):
                # rolling bf16 x half-slab for the v projection (2KB pieces)
                xs = xslpool.tile(
                    [128, KD, HS], BF16, tag="xsl", name=f"x_{b}_{hn}"
                )
                for t in range(KD):
                    nc.sync.dma_start(
                        out=xs[:, t, :],
                        in_=xT[b, 128 * t : 128 * (t + 1), HS * hn : HS * (hn + 1)],
                    )
                x_sb[(b, hn)] = xs

            load_x8slab(0, 0)
            load_xslab(0, 0)

            ones_sb = persist.tile([1, 128], BF16)
            nc.vector.memset(ones_sb, 1.0)


            a2a_in = {
                b: dram.tile([NCORES, 65, 2, HSL], BF16, name=f"a2a_in_{b}")
                for b in range(B)
            }
            a2a_out = {
                b: dram.tile([NCORES, 65, 2, HSL], BF16, name=f"a2a_out_{b}")
                for b in range(B)
            }
            qkvT = {}
            vp8 = {}

            def _qkv_dst(b):
                if b not in qkvT:
                    qkvT[b] = qkpool.tile(
                        [128, 3, S], BF16, tag="qkvT", name=f"qkvT_{b}"
                    )
                return qkvT[b]

            prog_qk = set()
            prog_vu = {0: 0, 1: 0}

            def emit_qk_group(b, w, n):
                # fp8 DoubleRow: contraction pairs packed, 2x PE throughput
                dst = _qkv_dst(b)[:, w, SQ * n : SQ * (n + 1)]
                ps = ps_mix.tile([128, SQ], F32, tag="mix", name=f"psqk_{b}_{w}_{n}")
                for tt in range(KT):
                    nc.tensor.matmul(
                        ps,
                        wqk8_sb[:, w, tt, :, :],
                        x8_sb[b][tt][:, :, SQ * n : SQ * (n + 1)],
                        start=(tt == 0),
                        stop=(tt == KT - 1),
                        perf_mode=DoubleRow,
                    )
                nc.vector.tensor_copy(dst, ps)
                if w == 1:
                    prog_qk.add((b, n))

            def emit_v_group(b, n):
                # v projection in bf16 (values need the precision)
                dst = _qkv_dst(b)[:, 2, SQ * n : SQ * (n + 1)]
                ps = ps_mix.tile([128, SQ], F32, tag="mix", name=f"psv_{b}_{n}")
                xs = x_sb[(b, n // CPH)]
                c0 = (n % CPH) * SQ
                for t in range(KD):
                    nc.tensor.matmul(
                        ps,
                        wv_sb[:, t, :],
                        xs[:, t, c0 : c0 + SQ],
                        start=(t == 0),
                        stop=(t == KD - 1),
                    )
                nc.vector.tensor_copy(dst, ps)

            vp0 = {}

            def emit_v_unit(b, i):
                # transpose key-tile i of v into the fp8 DoubleRow pair layout
                # vp8[p, pair, kt, hs, m] (m stride padded to 72 for step%16);
                # key tiles 0-3 also get a bf16 copy for the chunk-0 AV path
                if b not in vp8:
                    vp8[b] = vppool.tile(
                        [128, NT // 2, 2, 2, 72], F8E4, tag="vp", name=f"vp_{b}"
                    )
                    nc.vector.memset(vp8[b][:, :, :, :, 64], 1.0)
                    vp0[b] = vppool.tile(
                        [128, 4, 2, 65], BF16, tag="vp0", name=f"vp0_{b}"
                    )
                    nc.vector.memset(vp0[b][:, :, :, 64], 1.0)
                pst = ps_mix.tile([128, 128], BF16, tag="mix", name=f"psvt_{b}_{i}")
                nc.tensor.transpose(
                    pst, qkvT[b][:, 2, 128 * i : 128 * (i + 1)], ident[:, :]
                )
                for hs in range(2):
                    nc.vector.tensor_copy(
                        vp8[b][:, i // 2, i % 2, hs, 0:64],
                        pst[:, 64 * hs : 64 * hs + 64],
                    )
                    if i < 4:
                        nc.vector.tensor_copy(
                            vp0[b][:, i, hs, 0:64],
                            pst[:, 64 * hs : 64 * hs + 64],
                        )
                prog_vu[b] += 1

            # ---- attention tick machinery ----
            att_state = {}

            def attn_tick(b, n, j):
                """Scores for key-tile j of chunk (b, n), both heads packed as
                2-way row-tiled K=64 matmuls, then one exp over both heads."""
                st = att_state.setdefault((b, n), {"at": {}, "ot": {}})
                p = j // 2
                off = max(0, 128 * j - SQ * n)
                fp8 = n > 0  # chunk 0 runs bf16 AV (few keys: fp8 at too lossy)
                if j % 2 == 0:
                    if fp8:
                        at8 = atpool.tile(
                            [128, 2, 2, SQ], F8E4, tag="at", name=f"at_{b}_{n}_{p}"
                        )
                        off_odd = max(0, 128 * (j + 1) - SQ * n)
                        if off_odd > off:
                            # odd key-tile's dead zone: out-of-causal, zero it
                            nc.vector.memset(at8[:, 1, :, off:off_odd], 0.0)
                    else:
                        at8 = at16pool.tile(
                            [128, 2, 2, SQ], BF16, tag="at16", name=f"at_{b}_{n}_{p}"
                        )
                    st["at"][p] = at8
                else:
                    at8 = st["at"][p]
                sc = ps_sc.tile([128, 2, SQ], F32, tag="sc", name=f"sc_{b}_{n}_{j}")
                for h in range(2):
                    nc.tensor.matmul(
                        sc[:, h, off:],
                        qkvT[b][64 * h : 64 * h + 64, 1, 128 * j : 128 * (j + 1)],
                        qkvT[b][64 * h : 64 * h + 64, 0, SQ * n + off : SQ * (n + 1)],
                        start=True,
                        stop=True,
                    )
                nc.scalar.activation(
                    at8[:, j % 2, :, off:],
                    sc[:, :, off:],
                    Exp,
                    scale=SCALE / (QK_PRESCALE * QK_PRESCALE),
                )
                if j >= 4 * n:
                    # diagonal block: zero the strict upper triangle
                    nc.vector.tensor_mul(
                        at8[:, j % 2, :, off : off + 128],
                        at8[:, j % 2, :, off : off + 128],
                        mask_sb if fp8 else mask_sb16,
                    )

            def av_pair(b, n, p, jmax):
                """AV for key-tile pair p of chunk (b, n): one fp8 DoubleRow
                matmul per head consuming both key tiles; den rides as row 64."""
                st = att_state[(b, n)]
                at8 = st["at"][p]
                off = max(0, 128 * 2 * p - SQ * n)
                last = 2 * p + 2 >= jmax
                for h in range(2):
                    if h not in st["ot"]:
                        st["ot"][h] = ps_oT.tile(
                            [65, SQ], F32, tag="ot", name=f"ot_{b}_{n}_{h}"
                        )
                    if n > 0:
                        nc.tensor.matmul(
                            st["ot"][h][:, off:],
                            vp8[b][:, p, :, h, 0:65],
                            at8[:, :, h, off:],
                            start=(p == 0),
                            stop=last,
                            perf_mode=DoubleRow,
                        )
                    else:
                        for k in range(2):
                            j = 2 * p + k
                            offk = max(0, 128 * j - SQ * n)
                            nc.tensor.matmul(
                                st["ot"][h][:, offk:],
                                vp0[b][:, j, h, 0:65],
                                at8[:, k, h, offk:],
                                start=(j == 0),
                                stop=(j == jmax - 1),
                            )
                if last:
                    # [65, dest, hs, HSL]: each dest's payload is contiguous
                    # per partition row (1KB pieces for the staging DMA)
                    DPC = SQ // HSL
                    oT = prjpool.tile(
                        [65, DPC, 2, HSL], BF16, tag="oT", name=f"oT_{b}_{n}"
                    )
                    for h in range(2):
                        nc.vector.tensor_copy(
                            oT[:, :, h, :],
                            st["ot"][h][:, :].rearrange("p (d s) -> p d s", d=DPC),
                        )
                    # staging for the dest cores this chunk completes
                    for d in range(n * DPC, (n + 1) * DPC):
                        nc.sync.dma_start(
                            out=a2a_in[b][d, :, :, :],
                            in_=oT[:, d - n * DPC, :, :],
                        )
                    if n == NCH - 1:
                        nc.gpsimd.collective_compute(
                            "AllToAll",
                            mybir.AluOpType.bypass,
                            replica_groups=[list(range(NCORES))],
                            ins=[a2a_in[b][:, :, :, :].opt()],
                            outs=[a2a_out[b][:, :, :, :].opt()],
                        )

            # ---- phase D (output projection) pieces for batch b ----
            dstate = {}

            def emit_D_head(b):
                st_ = dstate.setdefault(b, {})
                den = prjpool.tile([40, HSL], BF16, tag="den", name=f"den_{b}")
                den32 = prjpool.tile([40, HSL], F32, tag="den32", name=f"den32_{b}")
                rcp32 = prjpool.tile([40, HSL], F32, tag="rcp32", name=f"rcp32_{b}")
                rcp = prjpool.tile([40, HSL], F32R, tag="rcp", name=f"rcp_{b}")
                st_["rcp"] = rcp
                st_["onrm"] = prjpool.tile(
                    [128, KD, HSL], BF16, tag="onrm", name=f"onrm_{b}"
                )
                nc.vector.memset(den32, 1.0)
                for hs in range(2):
                    r0 = 32 * hs
                    nc.sync.dma_start(
                        out=den[r0 : r0 + KD, :], in_=a2a_out[b][:, 64, hs, :]
                    )
                    nc.vector.tensor_copy(
                        den32[r0 : r0 + KD, :], den[r0 : r0 + KD, :]
                    )
                with nc.allow_low_precision(reason="softmax denom recip"):
                    nc.vector.reciprocal_approx_fast(rcp32, den32)
                nc.vector.tensor_copy(rcp, rcp32)

            def emit_D_norm(b, t):
                st_ = dstate[b]
                for hs in range(2):
                    nc.sync.dma_start(
                        out=st_["onrm"][64 * hs : 64 * hs + 64, t, :],
                        in_=a2a_out[b][t, 0:64, hs, :],
                    )
                bc = ps_mix.tile([128, HSL], F32, tag="mix", name=f"bc_{b}_{t}")
                nc.tensor.matmul(
                    bc, sel_sb[:, t, :], st_["rcp"], start=True, stop=True
                )
                nc.vector.tensor_mul(
                    st_["onrm"][:, t, :], st_["onrm"][:, t, :], bc
                )

            def emit_D_group(b, st, nn, tail=False):
                st_ = dstate[b]
                acc = ps_mix.tile(
                    [128, 512], F32, tag="mix", name=f"acc_{b}_{st}_{nn}"
                )
                for t in range(KD):
                    nc.tensor.matmul(
                        acc,
                        st_["onrm"][:, t, 128 * st : 128 * (st + 1)],
                        wpT_sb[:, t, 512 * nn : 512 * (nn + 1)],
                        start=(t == 0),
                        stop=False,
                    )
                nc.tensor.matmul(
                    acc, ones_sb, bp_sb[:, 512 * nn : 512 * (nn + 1)],
                    start=False, stop=True,
                )
                yt = ypool.tile([128, 512], F32, tag="y", name=f"y_{b}_{st}_{nn}")
                if tail:
                    nc.scalar.copy(yt, acc)
                else:
                    nc.vector.tensor_copy(yt, acc)
                nc.sync.dma_start(
                    out=y[
                        b * HSL + 128 * st : b * HSL + 128 * (st + 1),
                        512 * nn : 512 * (nn + 1),
                    ],
                    in_=yt,
                )

            # ---- the unified pipeline ----
            # lead-in: minimum to start (b0, chunk0): qk+v group 0, v-units 0-3
            emit_qk_group(0, 0, 0)
            emit_qk_group(0, 1, 0)
            emit_v_group(0, 0)
            for i in range(4):
                emit_v_unit(0, i)

            # filler list: (gate_tick, closure), consumed in order
            fillers = []
            for n in range(1, NCH):
                if n == 1 and NCH > CPH:
                    fillers.append((0, lambda: load_x8slab(0, 1)))
                    fillers.append((0, lambda: load_xslab(0, 1)))
                fillers.append((0, lambda n=n: emit_qk_group(0, 0, n)))
                fillers.append((0, lambda n=n: emit_qk_group(0, 1, n)))
                fillers.append((0, lambda n=n: emit_v_group(0, n)))
                for i in range(4 * n, 4 * n + 4):
                    fillers.append((0, lambda i=i: emit_v_unit(0, i)))

            # tick index bookkeeping
            ticks_b0 = sum(4 * n + 4 for n in range(NCH))
            total_ticks = 2 * ticks_b0

            def _wp_loads():
                nc.sync.dma_start(out=wpT_sb, in_=WpT[:, :, :])
                nc.sync.dma_start(out=bp_sb, in_=bp[:, :])
                nc.sync.dma_start(out=sel_sb, in_=sel[:, :, :].bitcast(F32R))

            # b1 x loads are emitted via fillers too (DMA queue slots sit
            # behind b0's x), interleaved x8/bf16 per half so each lands
            # just before its consumers
            fillers.append((2, lambda: load_x8slab(1, 0)))
            fillers.append((2, lambda: load_xslab(1, 0)))
            fillers.append((3, _wp_loads))
            for n in range(NCH):
                fillers.append((4, lambda n=n: emit_qk_group(1, 0, n)))
                fillers.append((4, lambda n=n: emit_qk_group(1, 1, n)))
                if n == 0 and NCH > CPH:
                    fillers.append((4, lambda: load_x8slab(1, 1)))
                    fillers.append((4, lambda: load_xslab(1, 1)))
                fillers.append((6, lambda n=n: emit_v_group(1, n)))
                for i in range(4 * n, 4 * n + 4):
                    fillers.append((6, lambda i=i: emit_v_unit(1, i)))



            wpT_sb = persist.tile([128, KD, D], BF16)
            bp_sb = persist.tile([1, D], BF16)
            sel_sb = persist.tile([40, KD, 128], F32R)

            # pending AV pairs: issue each one tick after its exp completes
            pend = []
            tick_no = [0]

            def drain_until(cond):
                # force-emit fillers (in order) until cond() holds
                while not cond():
                    assert fi_[0] < len(fillers), "filler list exhausted"
                    fillers[fi_[0]][1]()
                    fi_[0] += 1

            def run_tick(b, n, j, jmax):
                i = tick_no[0]
                drain_until(lambda: (b, n) in prog_qk)
                attn_tick(b, n, j)
                # pace fillers
                _credit[0] = min(
                    _credit[0] + (len(fillers) - fi_[0]) / max(1, total_ticks - i),
                    3.0,
                )
                popped = 0
                while (
                    _credit[0] >= 1.0
                    and fi_[0] < len(fillers)
                    and fillers[fi_[0]][0] <= i
                ):
                    fillers[fi_[0]][1]()
                    fi_[0] += 1
                    popped += 1
                    _credit[0] -= 1.0
                if popped == 0:
                    warm(2)
                # consume one pending AV pair (stagger >= 1 tick after its exp)
                while pend and pend[0][3] <= i - 1:
                    bb, nn_, pp, _, jm = pend.pop(0)
                    drain_until(lambda: prog_vu[bb] >= 2 * pp + 2)
                    av_pair(bb, nn_, pp, jm)
                if j % 2 == 1:
                    pend.append((b, n, j // 2, i, jmax))
                tick_no[0] += 1

            _credit = [0.0]
            fi_ = [0]

            for b in range(B):
                for n in range(NCH):
                    jmax = 4 * n + 4
                    for j in range(jmax):
                        run_tick(b, n, j, jmax)

            # flush remaining AV pairs (the last chunk's tail) — this emits
            # b1's staging + collective trigger; nothing a2a-dependent may
            # precede it in the PE queue
            while pend:
                bb, nn_, pp, _, jm = pend.pop(0)
                drain_until(lambda: prog_vu[bb] >= 2 * pp + 2)
                av_pair(bb, nn_, pp, jm)
                warm(2)
            # any unconsumed fillers
            while fi_[0] < len(fillers):
                fillers[fi_[0]][1]()
                fi_[0] += 1

            # phase D for batch 0 (its collective fired at the b0/b1 tick
            # boundary and has long landed); then batch 1 behind its own
            # collective, with a warm bridge across the wait
            warm(60)
            emit_D_head(0)
            for t in range(KD):
                emit_D_norm(0, t)
                warm(3)
            for st in range(NST):
                for nn in range(NNN):
                    emit_D_group(0, st, nn)
                    warm(4)
            warm(120)
            emit_D_head(1)
            for t in range(KD):
                emit_D_norm(1, t)
                warm(3)
            for st in range(NST):
                for nn in range(NNN):
                    emit_D_group(1, st, nn, tail=True)
                    warm(4)

    nc.compile()
    return nc


_built = {}


def get_nc(S=2048):
    if S not in _built:
        _built[S] = build(S)
    return _built[S]


def prep_inputs(x, Wq, Wk, Wv, Wp, bp):
    """Host-side shard prep. Returns per-core input maps."""
    import ml_dtypes

    BF = ml_dtypes.bfloat16
    F8 = ml_dtypes.float8_e4m3fn
    x = np.ascontiguousarray(np.asarray(x, dtype=np.float32))
    Wq, Wk, Wv = (np.asarray(w, dtype=np.float32) for w in (Wq, Wk, Wv))
    Wp = np.asarray(Wp, dtype=np.float32)
    bp = np.asarray(bp, dtype=np.float32)
    xT32 = np.ascontiguousarray(x.transpose(0, 2, 1))
    xT = xT32.astype(BF)
    KD = D // 128
    KT = KD // 2
    S = x.shape[1]
    # fp8 x for DoubleRow projections: [b, tt, p, kt, s]
    xT8 = np.ascontiguousarray(
        xT32.reshape(x.shape[0], KT, 2, 128, S).transpose(0, 1, 3, 2, 4)
    ).astype(F8)
    # WpT pre-arranged for SBUF: [p, t, i] with row t*128+p of Wp.T
    WpT = np.ascontiguousarray(
        Wp.T.reshape(KD, 128, D).transpose(1, 0, 2)
    ).astype(BF)
    mask1 = np.triu(np.ones((128, 128), dtype=np.float32))
    mask2_32 = np.ascontiguousarray(np.stack([mask1, mask1], axis=1))
    mask2 = mask2_32.astype(F8)
    mask2b = mask2_32.astype(BF)
    idin = np.eye(128, dtype=np.float32).astype(BF)
    sel = np.zeros((40, KD, 128), dtype=np.float32)
    for t in range(KD):
        sel[t, t, 0:64] = 1.0           # head 2t     -> den row t
        sel[32 + t, t, 64:128] = 1.0    # head 2t + 1 -> den row 32 + t
    in_maps = []
    QKS = QK_PRESCALE
    for c in range(NCORES):
        h0 = 2 * c
        wqk = np.stack(
            [
                np.concatenate([Wq[h0], Wq[h0 + 1]], axis=1),
                np.concatenate([Wk[h0], Wk[h0 + 1]], axis=1),
            ]
        ) * QKS  # [2, D, 128]
        # DoubleRow layout: [p, w, tt, kt, m]
        wqk8 = np.ascontiguousarray(
            wqk.reshape(2, KT, 2, 128, 128).transpose(3, 0, 1, 2, 4)
        ).astype(F8)
        wv = np.concatenate([Wv[h0], Wv[h0 + 1]], axis=1)  # [D, 128]
        wv = np.ascontiguousarray(
            wv.reshape(KD, 128, 128).transpose(1, 0, 2)
        ).astype(BF)  # [p, t, m]
        in_maps.append(
            {
                "xT8": xT8,
                "xT": xT,
                "Wqk8": wqk8,
                "Wv": wv,
                "WpT": WpT,
                "bp": bp.reshape(1, D).astype(BF),
                "mask2": mask2,
                "mask2b": mask2b,
                "idin": idin,
                "sel": sel,
            }
        )
    return in_maps


# inputs identical across cores are passed replicated (shipped once, not 8x)
_REPLICATED = {"xT8", "xT", "WpT", "bp", "mask2", "mask2b", "idin", "sel"}

_runners = {}


def _get_runner(S):
    """Cached jitted SPMD callable for the built module."""
    if S in _runners:
        return _runners[S]
    import jax
    import concourse.mybir as _mybir
    from concourse import bass2jax
    from jax.experimental.shard_map import shard_map
    from jax.sharding import Mesh, PartitionSpec

    nc = get_nc(S)
    bass2jax.install_neuronx_cc_hook()

    in_names, out_names, out_avals = [], [], []
    partition_name = nc.partition_id_tensor.name if nc.partition_id_tensor else None
    for alloc in nc.m.functions[0].allocations:
        if not isinstance(alloc, _mybir.MemoryLocationSet):
            continue
        name = alloc.memorylocations[0].name
        if alloc.kind == "ExternalInput":
            if name != partition_name:
                in_names.append(name)
        elif alloc.kind == "ExternalOutput":
            out_names.append(name)
            out_avals.append(
                jax.core.ShapedArray(tuple(alloc.tensor_shape), _mybir.dt.np(alloc.dtype))
            )
    n_params = len(in_names)
    all_in_names = list(in_names) + list(out_names)
    if partition_name is not None:
        all_in_names.append(partition_name)

    def _body(*args):
        operands = list(args)
        if partition_name is not None:
            operands.append(bass2jax.partition_id_tensor())
        outs = bass2jax._bass_exec_p.bind(
            *operands,
            out_avals=tuple(out_avals),
            in_names=tuple(all_in_names),
            out_names=tuple(out_names),
            lowering_input_output_aliases=(),
            sim_require_finite=True,
            sim_require_nnan=True,
            nc=nc,
        )
        return tuple(outs)

    devices = jax.devices()[:NCORES]
    mesh = Mesh(np.asarray(devices), ("core",))
    in_specs = tuple(
        PartitionSpec() if nm in _REPLICATED else PartitionSpec("core")
        for nm in in_names
    ) + (PartitionSpec("core"),) * len(out_names)
    out_specs = (PartitionSpec("core"),) * len(out_names)
    donate = tuple(range(n_params, n_params + len(out_names)))
    fn = jax.jit(
        shard_map(_body, mesh=mesh, in_specs=in_specs, out_specs=out_specs, check_rep=False),
        donate_argnums=donate,
        keep_unused=True,
    )
    r = (fn, in_names, out_names, out_avals, mesh)
    _runners[S] = r
    return r


class _Res:
    def __init__(self, results):
        self.results = results
        self.exec_time_ns = None


def run(x, Wq, Wk, Wv, Wp, bp, timings=None):
    import time as _time

    S = x.shape[1]
    t0 = _time.perf_counter()
    fn, in_names, out_names, out_avals, mesh = _get_runner(S)
    t1 = _time.perf_counter()
    in_maps = prep_inputs(x, Wq, Wk, Wv, Wp, bp)
    t2 = _time.perf_counter()
    args = []
    for nm in in_names:
        if nm in _REPLICATED:
            args.append(in_maps[0][nm])
        else:
            args.append(np.concatenate([in_maps[c][nm] for c in range(NCORES)], axis=0))
    zero_outs = [
        np.zeros((NCORES * av.shape[0], *av.shape[1:]), av.dtype) for av in out_avals
    ]
    t3 = _time.perf_counter()
    out_arrs = fn(*args, *zero_outs)
    out_np = [np.asarray(o) for o in out_arrs]
    t4 = _time.perf_counter()
    results = [
        {
            nm: out_np[i].reshape(NCORES, *out_avals[i].shape)[c]
            for i, nm in enumerate(out_names)
        }
        for c in range(NCORES)
    ]
    if timings is not None:
        timings.update(
            runner=t1 - t0, prep=t2 - t1, concat=t3 - t2, exec=t4 - t3
        )
    return _assemble_y([results[c]["y"] for c in range(NCORES)]), _Res(results)


def _assemble_y(per_core):
    """per-core y is [B*HSL, D]: rows [b*HSL:(b+1)*HSL] = batch b, s-slice c."""
    HSL = per_core[0].shape[0] // B
    S = HSL * NCORES
    out = np.empty((B, S, D), dtype=per_core[0].dtype)
    for c in range(NCORES):
        for b in range(B):
            out[b, HSL * c : HSL * (c + 1), :] = per_core[c][b * HSL : (b + 1) * HSL]
    return out


def kernel(x, Wq, Wk, Wv, Wp, bp):
    out, _ = run(x, Wq, Wk, Wv, Wp, bp)
    return out


# ---------------------------------------------------------------------------
# NTFF profiling support (test harness only; not needed for kernel()).
# ---------------------------------------------------------------------------

def _ntff_hook():
    import contextlib
    import ctypes

    lib = ctypes.CDLL("/opt/axon/libaxon_pjrt.so")
    lib.axon_start_nrt_profile.argtypes = [
        ctypes.POINTER(ctypes.c_int64),
        ctypes.c_size_t,
    ]
    lib.axon_start_nrt_profile.restype = ctypes.c_int64
    lib.axon_stop_nrt_profile.argtypes = [ctypes.c_char_p]
    lib.axon_stop_nrt_profile.restype = ctypes.c_int64

    @contextlib.contextmanager
    def _hook(output_dir, device_ids):
        import jax

        jax.devices()
        if device_ids:
            ids = (ctypes.c_int64 * len(device_ids))(*device_ids)
            rc = lib.axon_start_nrt_profile(ids, len(device_ids))
        else:
            rc = lib.axon_start_nrt_profile(None, 0)
        if rc != 0:
            raise RuntimeError(f"axon_start_nrt_profile rc={rc}")
        try:
            yield
        finally:
            n = lib.axon_stop_nrt_profile(str(output_dir).encode())
            print(f"profile: {n} file(s) written to {output_dir}")

    return _hook


def run_traced(x, Wq, Wk, Wv, Wp, bp, outdir=None, cores=(0,)):
    """Run once under NTFF profiling; returns (out, exec_time_ns, trace_path)."""
    import glob
    import tempfile

    import gauge.profiler
    from concourse._compat import FishPath

    S = x.shape[1]
    fn, in_names, out_names, out_avals, mesh = _get_runner(S)
    in_maps = prep_inputs(x, Wq, Wk, Wv, Wp, bp)
    args = []
    for nm in in_names:
        if nm in _REPLICATED:
            args.append(in_maps[0][nm])
        else:
            args.append(np.concatenate([in_maps[c][nm] for c in range(NCORES)], axis=0))
    zero_outs = [
        np.zeros((NCORES * av.shape[0], *av.shape[1:]), av.dtype) for av in out_avals
    ]
    # warm (compile + first exec)
    out_arrs = fn(*args, *zero_outs)
    _ = [np.asarray(o) for o in out_arrs]

    if outdir is None:
        outdir = tempfile.mkdtemp(prefix="ntff_")
    hook = _ntff_hook()
    zero_outs = [
        np.zeros((NCORES * av.shape[0], *av.shape[1:]), av.dtype) for av in out_avals
    ]
    with hook(outdir, list(cores)):
        out_arrs = fn(*args, *zero_outs)
        out_np = [np.asarray(o) for o in out_arrs]

    ntffs = glob.glob(f"{outdir}/*.ntff")
    if not ntffs:
        print(f"no NTFF files in {outdir}")
        return None, None, None
    nc = get_nc(S)
    profile = gauge.profiler.Profile(
        profile_path=FishPath(outdir),
        kernel_dev_mode=True,
        profile_on_exit=False,
        bass_kernel=nc.m,
        offline_processing=True,
        fname="*_body*",
        metadata={"artifacts_path": outdir},
    )
    results = profile.to_perfetto(model_index=tuple(range(len(cores))))
    exec_ns = max(r.exec_time_ns for r in results)
    yfull = _assemble_y(
        [out_np[out_names.index("y")].reshape(NCORES, -1, D)[c] for c in range(NCORES)]
    )
    return yfull, exec_ns, results[0].trace_path


# revision 56
# speedup vs baseline: 1.0393x; 1.0393x over previous
"""Trainium2 Bass kernel for causal multi-head attention + output projection.

Problem: B=2, S=2048, D=1024, H=16 heads of HD=64; fp32; causal softmax
scaled by D**-0.5; output projection with bias.

Sharding: 2 heads per core (tensor parallel on heads) for QKV + attention,
then an on-device AllToAll reshards from head-split to sequence-split and
each core computes its 512 rows of the output projection locally.

v3 schedule notes (vs v2 baseline):
 - single unified 80-tick pipeline: per tick both heads' score matmuls run
   concurrently as 2-way row-tiled K=64 matmuls (tile_position via base
   partitions 0/64), one exp per tick covers both heads.
 - AV uses fp8 DoubleRow over key-tile pairs (2 key tiles per matmul pass),
   with the softmax-denominator ones-row riding as output row 64.
 - all of x is fp8 (no bf16 copy): Q,K,V projections are all DoubleRow.
 - the two heads' A2A payloads are merged into one collective per batch.
 - QKV projection / V-transpose / phase-D work runs as paced fillers inside
   the attention tick stream, so the scalar-engine exp is never exposed.
"""

import sys

sys.path.insert(0, "/opt/trn_rl_repo")

import numpy as np

import concourse.bacc as bacc
import concourse.mybir as mybir
import concourse.tile as tile

B, D, H, HD = 2, 1024, 16, 64
NCORES = 8
SCALE = float(D) ** -0.5
QK_PRESCALE = 8.0  # Wq,Wk,Wv scaled by 8 before fp8 cast (avoids subnormals)
F32 = mybir.dt.float32
F32R = mybir.dt.float32r
BF16 = mybir.dt.bfloat16
F8E4 = mybir.dt.float8e4
DoubleRow = mybir.MatmulPerfMode.DoubleRow
Exp = mybir.ActivationFunctionType.Exp


def build(S=2048, dump=False):
    KD = D // 128          # contraction tiles for the projections (8)
    KT = KD // 2           # fp8 DoubleRow contraction pair-tiles (4)
    NT = S // 128          # key tiles
    SQ = 512               # query-chunk width
    NCH = S // SQ          # query chunks per (batch, head)
    HSL = S // NCORES      # rows of output owned per core per batch
    NST = HSL // 128       # 128-row tiles per core per batch
    NNN = D // 512         # 512-col output chunks

    nc = bacc.Bacc("TRN2", target_bir_lowering=False, debug=False)
    # fp8 x arranged for DoubleRow: [b, tt, p, kt, s] with d = 256*tt + 128*kt + p
    xT8 = nc.dram_tensor("xT8", [B, KT, 128, 2, S], F8E4, kind="ExternalInput")
    # q,k weights (x QK_PRESCALE) for DoubleRow: [p, w, tt, kt, m]
    Wqk8 = nc.dram_tensor("Wqk8", [128, 2, KT, 2, 128], F8E4, kind="ExternalInput")
    # bf16 x (for the v projection; fp8 x is too lossy for values)
    xT = nc.dram_tensor("xT", [B, D, S], BF16, kind="ExternalInput")
    Wv = nc.dram_tensor("Wv", [128, KD, 128], BF16, kind="ExternalInput")
    WpT = nc.dram_tensor("WpT", [128, KD, D], BF16, kind="ExternalInput")
    bp = nc.dram_tensor("bp", [1, D], BF16, kind="ExternalInput")
    # causal mask for the diagonal 128-block, replicated for the 2 heads
    mask2 = nc.dram_tensor("mask2", [128, 2, 128], F8E4, kind="ExternalInput")
    mask2b = nc.dram_tensor("mask2b", [128, 2, 128], BF16, kind="ExternalInput")
    idin = nc.dram_tensor("idin", [128, 128], BF16, kind="ExternalInput")
    sel = nc.dram_tensor("sel", [40, KD, 128], F32, kind="ExternalInput")
    # y rows: [0:HSL] = batch0 s-slice, [HSL:2*HSL] = batch1 s-slice
    y = nc.dram_tensor("y", [B * HSL, D], F32, kind="ExternalOutput")

    with tile.TileContext(nc) as tc:
        ctx_pools = [
            tc.tile_pool(name="persist", bufs=1),
            tc.tile_pool(name="dram", bufs=1, space="DRAM"),
            tc.tile_pool(name="wq", bufs=1),
            tc.tile_pool(name="xp", bufs=2),
            tc.tile_pool(name="qk", bufs=2),
            tc.tile_pool(name="vp", bufs=2),
            tc.tile_pool(name="at", bufs=4),
            tc.tile_pool(name="at16", bufs=2),
            tc.tile_pool(name="xsl", bufs=2),
            tc.tile_pool(name="prj", bufs=2),
            tc.tile_pool(name="yo", bufs=2),
            tc.tile_pool(name="ps_mix", bufs=2, space="PSUM"),
            tc.tile_pool(name="ps_sc", bufs=2, space="PSUM"),
            tc.tile_pool(name="ps_oT", bufs=2, space="PSUM"),
        ]
        import contextlib

        with contextlib.ExitStack() as stk:
            (
                persist, dram, wpool, xpool, qkpool, vppool, atpool,
                at16pool, xslpool,
                prjpool, ypool, ps_mix, ps_sc, ps_oT,
            ) = [stk.enter_context(p) for p in ctx_pools]

            # ---- critical-path first: small constants, weights, batch-0 x ----
            ident = persist.tile([128, 128], BF16)
            nc.sync.dma_start(out=ident, in_=idin[:, :])
            mask_sb = persist.tile([128, 2, 128], F8E4)
            nc.sync.dma_start(out=mask_sb, in_=mask2[:, :, :])
            mask_sb16 = persist.tile([128, 2, 128], BF16)
            nc.sync.dma_start(out=mask_sb16, in_=mask2b[:, :, :])
            wqk8_sb = wpool.tile([128, 2, KT, 2, 128], F8E4, tag="wqk8")
            nc.sync.dma_start(out=wqk8_sb, in_=Wqk8[:, :, :, :, :])
            wv_sb = wpool.tile([128, KD, 128], BF16, tag="wv")
            nc.sync.dma_start(out=wv_sb, in_=Wv[:, :, :])

            _warm_n = [0]

            def warm(k):
                # back-to-back thin matmuls: keeps the PE HAM activity window
                # busy across DMA / collective waits; 1-col stationary makes
                # the LDWEIGHTS cost negligible
                _warm_n[0] += 1
                wps = ps_mix.tile(
                    [128, SQ], F32, tag="mix", name=f"warm_{_warm_n[0]}"
                )
                for _ in range(k):
                    nc.tensor.matmul(
                        wps[0:1, 0:128], ident[:, 0:1], ident, start=True, stop=True
                    )

            warm(26)  # bridge until the first x slab lands

            x8_sb, x_sb = {}, {}

            HS = S // 2  # half-sequence slab width (bigger DMA pieces)
            CPH = HS // SQ  # query chunks per half-slab

            def load_x8slab(b, hn):
                # fp8 x half-slab hn for batch b (1KB contiguous pieces)
                if b not in x8_sb:
                    x8_sb[b] = [
                        xpool.tile(
                            [128, 2, S], F8E4, tag=f"x8{tt}", name=f"x8_{b}_{tt}"
                        )
                        for tt in range(KT)
                    ]
                # scalar HWDGE queue: runs parallel to the sync queue that
                # carries the (2x bigger) bf16 stream
                for tt in range(KT):
                    nc.scalar.dma_start(
                        out=x8_sb[b][tt][:, :, HS * hn : HS * (hn + 1)],
                        in_=xT8[b, tt, :, :, HS * hn : HS * (hn + 1)],
                    )

            def load_xslab(b, hn):
                # rolling bf16 x half-slab for the v projection (2KB pieces)
                xs = xslpool.tile(
                    [128, KD, HS], BF16, tag="xsl", name=f"x_{b}_{hn}"
                )
                for t in range(KD):
                    nc.sync.dma_start(
                        out=xs[:, t, :],
                        in_=xT[b, 128 * t : 128 * (t + 1), HS * hn : HS * (hn + 1)],
                    )
                x_sb[(b, hn)] = xs

            load_x8slab(0, 0)
            load_xslab(0, 0)

            ones_sb = persist.tile([1, 128], BF16)
            nc.vector.memset(ones_sb, 1.0)


            a2a_in = {
                b: dram.tile([NCORES, 65, 2, HSL], BF16, name=f"a2a_in_{b}")
                for b in range(B)
            }
            a2a_out = {
                b: dram.tile([NCORES, 65, 2, HSL], BF16, name=f"a2a_out_{b}")
                for b in range(B)
            }
            qkvT = {}
            vp8 = {}

            def _qkv_dst(b):
                if b not in qkvT:
                    qkvT[b] = qkpool.tile(
                        [128, 3, S], BF16, tag="qkvT", name=f"qkvT_{b}"
                    )
                return qkvT[b]

            prog_qk = set()
            prog_vu = {0: 0, 1: 0}

            def emit_qk_group(b, w, n):
                # fp8 DoubleRow: contraction pairs packed, 2x PE throughput
                dst = _qkv_dst(b)[:, w, SQ * n : SQ * (n + 1)]
                ps = ps_mix.tile([128, SQ], F32, tag="mix", name=f"psqk_{b}_{w}_{n}")
                for tt in range(KT):
                    nc.tensor.matmul(
                        ps,
                        wqk8_sb[:, w, tt, :, :],
                        x8_sb[b][tt][:, :, SQ * n : SQ * (n + 1)],
                        start=(tt == 0),
                        stop=(tt == KT - 1),
                        perf_mode=DoubleRow,
                    )
                nc.vector.tensor_copy(dst, ps)
                if w == 1:
                    prog_qk.add((b, n))

            def emit_v_group(b, n):
                # v projection in bf16 (values need the precision)
                dst = _qkv_dst(b)[:, 2, SQ * n : SQ * (n + 1)]
                ps = ps_mix.tile([128, SQ], F32, tag="mix", name=f"psv_{b}_{n}")
                xs = x_sb[(b, n // CPH)]
                c0 = (n % CPH) * SQ
                for t in range(KD):
                    nc.tensor.matmul(
                        ps,
                        wv_sb[:, t, :],
                        xs[:, t, c0 : c0 + SQ],
                        start=(t == 0),
                        stop=(t == KD - 1),
                    )
                nc.vector.tensor_copy(dst, ps)

            vp0 = {}

            def emit_v_unit(b, i):
                # transpose key-tile i of v into the fp8 DoubleRow pair layout
                # vp8[p, pair, kt, hs, m] (m stride padded to 72 for step%16);
                # key tiles 0-3 also get a bf16 copy for the chunk-0 AV path
                if b not in vp8:
                    vp8[b] = vppool.tile(
                        [128, NT // 2, 2, 2, 72], F8E4, tag="vp", name=f"vp_{b}"
                    )
                    nc.vector.memset(vp8[b][:, :, :, :, 64], 1.0)
                    vp0[b] = vppool.tile(
                        [128, 4, 2, 65], BF16, tag="vp0", name=f"vp0_{b}"
                    )
                    nc.vector.memset(vp0[b][:, :, :, 64], 1.0)
                pst = ps_mix.tile([128, 128], BF16, tag="mix", name=f"psvt_{b}_{i}")
                nc.tensor.transpose(
                    pst, qkvT[b][:, 2, 128 * i : 128 * (i + 1)], ident[:, :]
                )
                for hs in range(2):
                    nc.vector.tensor_copy(
                        vp8[b][:, i // 2, i % 2, hs, 0:64],
                        pst[:, 64 * hs : 64 * hs + 64],
                    )
                    if i < 4:
                        nc.vector.tensor_copy(
                            vp0[b][:, i, hs, 0:64],
                            pst[:, 64 * hs : 64 * hs + 64],
                        )
                prog_vu[b] += 1

            # ---- attention tick machinery ----
            att_state = {}

            def attn_tick(b, n, j):
                """Scores for key-tile j of chunk (b, n), both heads packed as
                2-way row-tiled K=64 matmuls, then one exp over both heads."""
                st = att_state.setdefault((b, n), {"at": {}, "ot": {}})
                p = j // 2
                off = max(0, 128 * j - SQ * n)
                fp8 = n > 0  # chunk 0 runs bf16 AV (few keys: fp8 at too lossy)
                if j % 2 == 0:
                    if fp8:
                        at8 = atpool.tile(
                            [128, 2, 2, SQ], F8E4, tag="at", name=f"at_{b}_{n}_{p}"
                        )
                        off_odd = max(0, 128 * (j + 1) - SQ * n)
                        if off_odd > off:
                            # odd key-tile's dead zone: out-of-causal, zero it
                            nc.vector.memset(at8[:, 1, :, off:off_odd], 0.0)
                    else:
                        at8 = at16pool.tile(
                            [128, 2, 2, SQ], BF16, tag="at16", name=f"at_{b}_{n}_{p}"
                        )
                    st["at"][p] = at8
                else:
                    at8 = st["at"][p]
                sc = ps_sc.tile([128, 2, SQ], F32, tag="sc", name=f"sc_{b}_{n}_{j}")
                for h in range(2):
                    nc.tensor.matmul(
                        sc[:, h, off:],
                        qkvT[b][64 * h : 64 * h + 64, 1, 128 * j : 128 * (j + 1)],
                        qkvT[b][64 * h : 64 * h + 64, 0, SQ * n + off : SQ * (n + 1)],
                        start=True,
                        stop=True,
                    )
                nc.scalar.activation(
                    at8[:, j % 2, :, off:],
                    sc[:, :, off:],
                    Exp,
                    scale=SCALE / (QK_PRESCALE * QK_PRESCALE),
                )
                if j >= 4 * n:
                    # diagonal block: zero the strict upper triangle
                    nc.vector.tensor_mul(
                        at8[:, j % 2, :, off : off + 128],
                        at8[:, j % 2, :, off : off + 128],
                        mask_sb if fp8 else mask_sb16,
                    )

            def av_pair(b, n, p, jmax):
                """AV for key-tile pair p of chunk (b, n): one fp8 DoubleRow
                matmul per head consuming both key tiles; den rides as row 64."""
                st = att_state[(b, n)]
                at8 = st["at"][p]
                off = max(0, 128 * 2 * p - SQ * n)
                last = 2 * p + 2 >= jmax
                for h in range(2):
                    if h not in st["ot"]:
                        st["ot"][h] = ps_oT.tile(
                            [65, SQ], F32, tag="ot", name=f"ot_{b}_{n}_{h}"
                        )
                    if n > 0:
                        nc.tensor.matmul(
                            st["ot"][h][:, off:],
                            vp8[b][:, p, :, h, 0:65],
                            at8[:, :, h, off:],
                            start=(p == 0),
                            stop=last,
                            perf_mode=DoubleRow,
                        )
                    else:
                        for k in range(2):
                            j = 2 * p + k
                            offk = max(0, 128 * j - SQ * n)
                            nc.tensor.matmul(
                                st["ot"][h][:, offk:],
                                vp0[b][:, j, h, 0:65],
                                at8[:, k, h, offk:],
                                start=(j == 0),
                                stop=(j == jmax - 1),
                            )
                if last:
                    # [65, dest, hs, HSL]: each dest's payload is contiguous
                    # per partition row (1KB pieces for the staging DMA)
                    DPC = SQ // HSL
                    oT = prjpool.tile(
                        [65, DPC, 2, HSL], BF16, tag="oT", name=f"oT_{b}_{n}"
                    )
                    for h in range(2):
                        nc.vector.tensor_copy(
                            oT[:, :, h, :],
                            st["ot"][h][:, :].rearrange("p (d s) -> p d s", d=DPC),
                        )
                    # staging for the dest cores this chunk completes
                    for d in range(n * DPC, (n + 1) * DPC):
                        nc.sync.dma_start(
                            out=a2a_in[b][d, :, :, :],
                            in_=oT[:, d - n * DPC, :, :],
                        )
                    if n == NCH - 1:
                        nc.gpsimd.collective_compute(
                            "AllToAll",
                            mybir.AluOpType.bypass,
                            replica_groups=[list(range(NCORES))],
                            ins=[a2a_in[b][:, :, :, :].opt()],
                            outs=[a2a_out[b][:, :, :, :].opt()],
                        )

            # ---- phase D (output projection) pieces for batch b ----
            dstate = {}

            def emit_D_head(b):
                st_ = dstate.setdefault(b, {})
                den = prjpool.tile([40, HSL], BF16, tag="den", name=f"den_{b}")
                den32 = prjpool.tile([40, HSL], F32, tag="den32", name=f"den32_{b}")
                rcp32 = prjpool.tile([40, HSL], F32, tag="rcp32", name=f"rcp32_{b}")
                rcp = prjpool.tile([40, HSL], F32R, tag="rcp", name=f"rcp_{b}")
                st_["rcp"] = rcp
                st_["onrm"] = prjpool.tile(
                    [128, KD, HSL], BF16, tag="onrm", name=f"onrm_{b}"
                )
                nc.vector.memset(den32, 1.0)
                for hs in range(2):
                    r0 = 32 * hs
                    nc.sync.dma_start(
                        out=den[r0 : r0 + KD, :], in_=a2a_out[b][:, 64, hs, :]
                    )
                    nc.vector.tensor_copy(
                        den32[r0 : r0 + KD, :], den[r0 : r0 + KD, :]
                    )
                with nc.allow_low_precision(reason="softmax denom recip"):
                    nc.vector.reciprocal_approx_fast(rcp32, den32)
                nc.vector.tensor_copy(rcp, rcp32)

            def emit_D_norm(b, t):
                st_ = dstate[b]
                for hs in range(2):
                    nc.sync.dma_start(
                        out=st_["onrm"][64 * hs : 64 * hs + 64, t, :],
                        in_=a2a_out[b][t, 0:64, hs, :],
                    )
                bc = ps_mix.tile([128, HSL], F32, tag="mix", name=f"bc_{b}_{t}")
                nc.tensor.matmul(
                    bc, sel_sb[:, t, :], st_["rcp"], start=True, stop=True
                )
                nc.vector.tensor_mul(
                    st_["onrm"][:, t, :], st_["onrm"][:, t, :], bc
                )

            def emit_D_group(b, st, nn, tail=False):
                st_ = dstate[b]
                acc = ps_mix.tile(
                    [128, 512], F32, tag="mix", name=f"acc_{b}_{st}_{nn}"
                )
                for t in range(KD):
                    nc.tensor.matmul(
                        acc,
                        st_["onrm"][:, t, 128 * st : 128 * (st + 1)],
                        wpT_sb[:, t, 512 * nn : 512 * (nn + 1)],
                        start=(t == 0),
                        stop=False,
                    )
                nc.tensor.matmul(
                    acc, ones_sb, bp_sb[:, 512 * nn : 512 * (nn + 1)],
                    start=False, stop=True,
                )
                yt = ypool.tile([128, 512], F32, tag="y", name=f"y_{b}_{st}_{nn}")
                if tail:
                    nc.scalar.copy(yt, acc)
                else:
                    nc.vector.tensor_copy(yt, acc)
                nc.sync.dma_start(
                    out=y[
                        b * HSL + 128 * st : b * HSL + 128 * (st + 1),
                        512 * nn : 512 * (nn + 1),
                    ],
                    in_=yt,
                )

            # ---- the unified pipeline ----
            # lead-in: minimum to start (b0, chunk0): qk+v group 0, v-units 0-3
            emit_qk_group(0, 0, 0)
            emit_qk_group(0, 1, 0)
            emit_v_group(0, 0)
            for i in range(4):
                emit_v_unit(0, i)

            # filler list: (gate_tick, closure), consumed in order
            fillers = []
            for n in range(1, NCH):
                if n == 1 and NCH > CPH:
                    fillers.append((0, lambda: load_x8slab(0, 1)))
                    fillers.append((0, lambda: load_xslab(0, 1)))
                fillers.append((0, lambda n=n: emit_qk_group(0, 0, n)))
                fillers.append((0, lambda n=n: emit_qk_group(0, 1, n)))
                fillers.append((0, lambda n=n: emit_v_group(0, n)))
                for i in range(4 * n, 4 * n + 4):
                    fillers.append((0, lambda i=i: emit_v_unit(0, i)))

            # tick index bookkeeping
            ticks_b0 = sum(4 * n + 4 for n in range(NCH))
            total_ticks = 2 * ticks_b0

            def _wp_loads():
                nc.sync.dma_start(out=wpT_sb, in_=WpT[:, :, :])
                nc.sync.dma_start(out=bp_sb, in_=bp[:, :])
                nc.sync.dma_start(out=sel_sb, in_=sel[:, :, :].bitcast(F32R))

            # b1 x loads are emitted via fillers too (DMA queue slots sit
            # behind b0's x), interleaved x8/bf16 per half so each lands
            # just before its consumers
            fillers.append((2, lambda: load_x8slab(1, 0)))
            fillers.append((2, lambda: load_xslab(1, 0)))
            fillers.append((3, _wp_loads))
            # gate b1's projection work to just-before-use so it fills b1's
            # own (otherwise empty) ticks instead of stretching b0's phase;
            # chunk n of b1 starts at tick ticks_b0 + T0(n)
            T0 = [sum(4 * m + 4 for m in range(n)) for n in range(NCH)]
            for n in range(NCH):
                g_qk = 4 if n == 0 else max(4, ticks_b0 + T0[n] - 8)
                fillers.append((g_qk, lambda n=n: emit_qk_group(1, 0, n)))
                fillers.append((g_qk, lambda n=n: emit_qk_group(1, 1, n)))
                if n == 0 and NCH > CPH:
                    fillers.append((6, lambda: load_x8slab(1, 1)))
                    fillers.append((6, lambda: load_xslab(1, 1)))
                g_v = 6 if n == 0 else max(6, ticks_b0 + T0[n] - 6)
                fillers.append((g_v, lambda n=n: emit_v_group(1, n)))
                for i in range(4 * n, 4 * n + 4):
                    fillers.append((g_v, lambda i=i: emit_v_unit(1, i)))



            wpT_sb = persist.tile([128, KD, D], BF16)
            bp_sb = persist.tile([1, D], BF16)
            sel_sb = persist.tile([40, KD, 128], F32R)

            # pending AV pairs: issue each one tick after its exp completes
            pend = []
            tick_no = [0]

            def drain_until(cond):
                # force-emit fillers (in order) until cond() holds
                while not cond():
                    assert fi_[0] < len(fillers), "filler list exhausted"
                    fillers[fi_[0]][1]()
                    fi_[0] += 1

            def run_tick(b, n, j, jmax):
                i = tick_no[0]
                drain_until(lambda: (b, n) in prog_qk)
                attn_tick(b, n, j)
                # pace fillers
                _credit[0] = min(
                    _credit[0] + (len(fillers) - fi_[0]) / max(1, total_ticks - i),
                    3.0,
                )
                popped = 0
                while (
                    _credit[0] >= 1.0
                    and fi_[0] < len(fillers)
                    and fillers[fi_[0]][0] <= i
                ):
                    fillers[fi_[0]][1]()
                    fi_[0] += 1
                    popped += 1
                    _credit[0] -= 1.0
                if popped == 0:
                    warm(2)
                # consume one pending AV pair (stagger >= 1 tick after its exp)
                while pend and pend[0][3] <= i - 1:
                    bb, nn_, pp, _, jm = pend.pop(0)
                    drain_until(lambda: prog_vu[bb] >= 2 * pp + 2)
                    av_pair(bb, nn_, pp, jm)
                if j % 2 == 1:
                    pend.append((b, n, j // 2, i, jmax))
                tick_no[0] += 1

            _credit = [0.0]
            fi_ = [0]

            for b in range(B):
                for n in range(NCH):
                    jmax = 4 * n + 4
                    for j in range(jmax):
                        run_tick(b, n, j, jmax)

            # flush remaining AV pairs (the last chunk's tail) — this emits
            # b1's staging + collective trigger; nothing a2a-dependent may
            # precede it in the PE queue
            while pend:
                bb, nn_, pp, _, jm = pend.pop(0)
                drain_until(lambda: prog_vu[bb] >= 2 * pp + 2)
                av_pair(bb, nn_, pp, jm)
                warm(2)
            # any unconsumed fillers
            while fi_[0] < len(fillers):
                fillers[fi_[0]][1]()
                fi_[0] += 1

            # phase D for batch 0 (its collective fired at the b0/b1 tick
            # boundary and has long landed); then batch 1 behind its own
            # collective, with a warm bridge across the wait
            warm(60)
            emit_D_head(0)
            for t in range(KD):
                emit_D_norm(0, t)
                warm(3)
            for st in range(NST):
                for nn in range(NNN):
                    emit_D_group(0, st, nn)
                    warm(4)
            warm(120)
            emit_D_head(1)
            for t in range(KD):
                emit_D_norm(1, t)
                warm(3)
            for st in range(NST):
                for nn in range(NNN):
                    emit_D_group(1, st, nn, tail=True)
                    warm(4)

    nc.compile()
    return nc


_built = {}


def get_nc(S=2048):
    if S not in _built:
        _built[S] = build(S)
    return _built[S]


def prep_inputs(x, Wq, Wk, Wv, Wp, bp):
    """Host-side shard prep. Returns per-core input maps."""
    import ml_dtypes

    BF = ml_dtypes.bfloat16
    F8 = ml_dtypes.float8_e4m3fn
    x = np.ascontiguousarray(np.asarray(x, dtype=np.float32))
    Wq, Wk, Wv = (np.asarray(w, dtype=np.float32) for w in (Wq, Wk, Wv))
    Wp = np.asarray(Wp, dtype=np.float32)
    bp = np.asarray(bp, dtype=np.float32)
    xT32 = np.ascontiguousarray(x.transpose(0, 2, 1))
    xT = xT32.astype(BF)
    KD = D // 128
    KT = KD // 2
    S = x.shape[1]
    # fp8 x for DoubleRow projections: [b, tt, p, kt, s]
    xT8 = np.ascontiguousarray(
        xT32.reshape(x.shape[0], KT, 2, 128, S).transpose(0, 1, 3, 2, 4)
    ).astype(F8)
    # WpT pre-arranged for SBUF: [p, t, i] with row t*128+p of Wp.T
    WpT = np.ascontiguousarray(
        Wp.T.reshape(KD, 128, D).transpose(1, 0, 2)
    ).astype(BF)
    mask1 = np.triu(np.ones((128, 128), dtype=np.float32))
    mask2_32 = np.ascontiguousarray(np.stack([mask1, mask1], axis=1))
    mask2 = mask2_32.astype(F8)
    mask2b = mask2_32.astype(BF)
    idin = np.eye(128, dtype=np.float32).astype(BF)
    sel = np.zeros((40, KD, 128), dtype=np.float32)
    for t in range(KD):
        sel[t, t, 0:64] = 1.0           # head 2t     -> den row t
        sel[32 + t, t, 64:128] = 1.0    # head 2t + 1 -> den row 32 + t
    in_maps = []
    QKS = QK_PRESCALE
    for c in range(NCORES):
        h0 = 2 * c
        wqk = np.stack(
            [
                np.concatenate([Wq[h0], Wq[h0 + 1]], axis=1),
                np.concatenate([Wk[h0], Wk[h0 + 1]], axis=1),
            ]
        ) * QKS  # [2, D, 128]
        # DoubleRow layout: [p, w, tt, kt, m]
        wqk8 = np.ascontiguousarray(
            wqk.reshape(2, KT, 2, 128, 128).transpose(3, 0, 1, 2, 4)
        ).astype(F8)
        wv = np.concatenate([Wv[h0], Wv[h0 + 1]], axis=1)  # [D, 128]
        wv = np.ascontiguousarray(
            wv.reshape(KD, 128, 128).transpose(1, 0, 2)
        ).astype(BF)  # [p, t, m]
        in_maps.append(
            {
                "xT8": xT8,
                "xT": xT,
                "Wqk8": wqk8,
                "Wv": wv,
                "WpT": WpT,
                "bp": bp.reshape(1, D).astype(BF),
                "mask2": mask2,
                "mask2b": mask2b,
                "idin": idin,
                "sel": sel,
            }
        )
    return in_maps


# inputs identical across cores are passed replicated (shipped once, not 8x)
_REPLICATED = {"xT8", "xT", "WpT", "bp", "mask2", "mask2b", "idin", "sel"}

_runners = {}


def _get_runner(S):
    """Cached jitted SPMD callable for the built module."""
    if S in _runners:
        return _runners[S]
    import jax
    import concourse.mybir as _mybir
    from concourse import bass2jax
    from jax.experimental.shard_map import shard_map
    from jax.sharding import Mesh, PartitionSpec

    nc = get_nc(S)
    bass2jax.install_neuronx_cc_hook()

    in_names, out_names, out_avals = [], [], []
    partition_name = nc.partition_id_tensor.name if nc.partition_id_tensor else None
    for alloc in nc.m.functions[0].allocations:
        if not isinstance(alloc, _mybir.MemoryLocationSet):
            continue
        name = alloc.memorylocations[0].name
        if alloc.kind == "ExternalInput":
            if name != partition_name:
                in_names.append(name)
        elif alloc.kind == "ExternalOutput":
            out_names.append(name)
            out_avals.append(
                jax.core.ShapedArray(tuple(alloc.tensor_shape), _mybir.dt.np(alloc.dtype))
            )
    n_params = len(in_names)
    all_in_names = list(in_names) + list(out_names)
    if partition_name is not None:
        all_in_names.append(partition_name)

    def _body(*args):
        operands = list(args)
        if partition_name is not None:
            operands.append(bass2jax.partition_id_tensor())
        outs = bass2jax._bass_exec_p.bind(
            *operands,
            out_avals=tuple(out_avals),
            in_names=tuple(all_in_names),
            out_names=tuple(out_names),
            lowering_input_output_aliases=(),
            sim_require_finite=True,
            sim_require_nnan=True,
            nc=nc,
        )
        return tuple(outs)

    devices = jax.devices()[:NCORES]
    mesh = Mesh(np.asarray(devices), ("core",))
    in_specs = tuple(
        PartitionSpec() if nm in _REPLICATED else PartitionSpec("core")
        for nm in in_names
    ) + (PartitionSpec("core"),) * len(out_names)
    out_specs = (PartitionSpec("core"),) * len(out_names)
    donate = tuple(range(n_params, n_params + len(out_names)))
    fn = jax.jit(
        shard_map(_body, mesh=mesh, in_specs=in_specs, out_specs=out_specs, check_rep=False),
        donate_argnums=donate,
        keep_unused=True,
    )
    r = (fn, in_names, out_names, out_avals, mesh)
    _runners[S] = r
    return r


class _Res:
    def __init__(self, results):
        self.results = results
        self.exec_time_ns = None


def run(x, Wq, Wk, Wv, Wp, bp, timings=None):
    import time as _time

    S = x.shape[1]
    t0 = _time.perf_counter()
    fn, in_names, out_names, out_avals, mesh = _get_runner(S)
    t1 = _time.perf_counter()
    in_maps = prep_inputs(x, Wq, Wk, Wv, Wp, bp)
    t2 = _time.perf_counter()
    args = []
    for nm in in_names:
        if nm in _REPLICATED:
            args.append(in_maps[0][nm])
        else:
            args.append(np.concatenate([in_maps[c][nm] for c in range(NCORES)], axis=0))
    zero_outs = [
        np.zeros((NCORES * av.shape[0], *av.shape[1:]), av.dtype) for av in out_avals
    ]
    t3 = _time.perf_counter()
    out_arrs = fn(*args, *zero_outs)
    out_np = [np.asarray(o) for o in out_arrs]
    t4 = _time.perf_counter()
    results = [
        {
            nm: out_np[i].reshape(NCORES, *out_avals[i].shape)[c]
            for i, nm in enumerate(out_names)
        }
        for c in range(NCORES)
    ]
    if timings is not None:
        timings.update(
            runner=t1 - t0, prep=t2 - t1, concat=t3 - t2, exec=t4 - t3
        )
    return _assemble_y([results[c]["y"] for c in range(NCORES)]), _Res(results)


def _assemble_y(per_core):
    """per-core y is [B*HSL, D]: rows [b*HSL:(b+1)*HSL] = batch b, s-slice c."""
    HSL = per_core[0].shape[0] // B
    S = HSL * NCORES
    out = np.empty((B, S, D), dtype=per_core[0].dtype)
    for c in range(NCORES):
        for b in range(B):
            out[b, HSL * c : HSL * (c + 1), :] = per_core[c][b * HSL : (b + 1) * HSL]
    return out


def kernel(x, Wq, Wk, Wv, Wp, bp):
    out, _ = run(x, Wq, Wk, Wv, Wp, bp)
    return out


# ---------------------------------------------------------------------------
# NTFF profiling support (test harness only; not needed for kernel()).
# ---------------------------------------------------------------------------

def _ntff_hook():
    import contextlib
    import ctypes

    lib = ctypes.CDLL("/opt/axon/libaxon_pjrt.so")
    lib.axon_start_nrt_profile.argtypes = [
        ctypes.POINTER(ctypes.c_int64),
        ctypes.c_size_t,
    ]
    lib.axon_start_nrt_profile.restype = ctypes.c_int64
    lib.axon_stop_nrt_profile.argtypes = [ctypes.c_char_p]
    lib.axon_stop_nrt_profile.restype = ctypes.c_int64

    @contextlib.contextmanager
    def _hook(output_dir, device_ids):
        import jax

        jax.devices()
        if device_ids:
            ids = (ctypes.c_int64 * len(device_ids))(*device_ids)
            rc = lib.axon_start_nrt_profile(ids, len(device_ids))
        else:
            rc = lib.axon_start_nrt_profile(None, 0)
        if rc != 0:
            raise RuntimeError(f"axon_start_nrt_profile rc={rc}")
        try:
            yield
        finally:
            n = lib.axon_stop_nrt_profile(str(output_dir).encode())
            print(f"profile: {n} file(s) written to {output_dir}")

    return _hook


def run_traced(x, Wq, Wk, Wv, Wp, bp, outdir=None, cores=(0,)):
    """Run once under NTFF profiling; returns (out, exec_time_ns, trace_path)."""
    import glob
    import tempfile

    import gauge.profiler
    from concourse._compat import FishPath

    S = x.shape[1]
    fn, in_names, out_names, out_avals, mesh = _get_runner(S)
    in_maps = prep_inputs(x, Wq, Wk, Wv, Wp, bp)
    args = []
    for nm in in_names:
        if nm in _REPLICATED:
            args.append(in_maps[0][nm])
        else:
            args.append(np.concatenate([in_maps[c][nm] for c in range(NCORES)], axis=0))
    zero_outs = [
        np.zeros((NCORES * av.shape[0], *av.shape[1:]), av.dtype) for av in out_avals
    ]
    # warm (compile + first exec)
    out_arrs = fn(*args, *zero_outs)
    _ = [np.asarray(o) for o in out_arrs]

    if outdir is None:
        outdir = tempfile.mkdtemp(prefix="ntff_")
    hook = _ntff_hook()
    zero_outs = [
        np.zeros((NCORES * av.shape[0], *av.shape[1:]), av.dtype) for av in out_avals
    ]
    with hook(outdir, list(cores)):
        out_arrs = fn(*args, *zero_outs)
        out_np = [np.asarray(o) for o in out_arrs]

    ntffs = glob.glob(f"{outdir}/*.ntff")
    if not ntffs:
        print(f"no NTFF files in {outdir}")
        return None, None, None
    nc = get_nc(S)
    profile = gauge.profiler.Profile(
        profile_path=FishPath(outdir),
        kernel_dev_mode=True,
        profile_on_exit=False,
        bass_kernel=nc.m,
        offline_processing=True,
        fname="*_body*",
        metadata={"artifacts_path": outdir},
    )
    results = profile.to_perfetto(model_index=tuple(range(len(cores))))
    exec_ns = max(r.exec_time_ns for r in results)
    yfull = _assemble_y(
        [out_np[out_names.index("y")].reshape(NCORES, -1, D)[c] for c in range(NCORES)]
    )
    return yfull, exec_ns, results[0].trace_path


# revision 58
# speedup vs baseline: 1.1047x; 1.0629x over previous
"""Trainium2 Bass kernel for causal multi-head attention + output projection.

Problem: B=2, S=2048, D=1024, H=16 heads of HD=64; fp32; causal softmax
scaled by D**-0.5; output projection with bias.

Sharding: 2 heads per core (tensor parallel on heads) for QKV + attention,
then an on-device AllToAll reshards from head-split to sequence-split and
each core computes its 512 rows of the output projection locally.

v3 schedule notes (vs v2 baseline):
 - single unified 80-tick pipeline: per tick both heads' score matmuls run
   concurrently as 2-way row-tiled K=64 matmuls (tile_position via base
   partitions 0/64), one exp per tick covers both heads.
 - AV uses fp8 DoubleRow over key-tile pairs (2 key tiles per matmul pass),
   with the softmax-denominator ones-row riding as output row 64.
 - all of x is fp8 (no bf16 copy): Q,K,V projections are all DoubleRow.
 - the two heads' A2A payloads are merged into one collective per batch.
 - QKV projection / V-transpose / phase-D work runs as paced fillers inside
   the attention tick stream, so the scalar-engine exp is never exposed.
"""

import sys

sys.path.insert(0, "/opt/trn_rl_repo")

import numpy as np

import concourse.bacc as bacc
import concourse.mybir as mybir
import concourse.tile as tile

B, D, H, HD = 2, 1024, 16, 64
NCORES = 8
SCALE = float(D) ** -0.5
QK_PRESCALE = 8.0  # Wq,Wk,Wv scaled by 8 before fp8 cast (avoids subnormals)
F32 = mybir.dt.float32
F32R = mybir.dt.float32r
BF16 = mybir.dt.bfloat16
F8E4 = mybir.dt.float8e4
DoubleRow = mybir.MatmulPerfMode.DoubleRow
Exp = mybir.ActivationFunctionType.Exp


def build(S=2048, dump=False):
    KD = D // 128          # contraction tiles for the projections (8)
    KT = KD // 2           # fp8 DoubleRow contraction pair-tiles (4)
    NT = S // 128          # key tiles
    SQ = 512               # query-chunk width
    NCH = S // SQ          # query chunks per (batch, head)
    HSL = S // NCORES      # rows of output owned per core per batch
    NST = HSL // 128       # 128-row tiles per core per batch
    NNN = D // 512         # 512-col output chunks

    nc = bacc.Bacc("TRN2", target_bir_lowering=False, debug=False)
    # fp8 x arranged for DoubleRow: [b, tt, p, kt, s] with d = 256*tt + 128*kt + p
    xT8 = nc.dram_tensor("xT8", [B, KT, 128, 2, S], F8E4, kind="ExternalInput")
    # q,k weights (x QK_PRESCALE) for DoubleRow: [p, w, tt, kt, m]
    Wqk8 = nc.dram_tensor("Wqk8", [128, 2, KT, 2, 128], F8E4, kind="ExternalInput")
    # bf16 x (for the v projection; fp8 x is too lossy for values)
    xT = nc.dram_tensor("xT", [B, D, S], BF16, kind="ExternalInput")
    Wv = nc.dram_tensor("Wv", [128, KD, 128], BF16, kind="ExternalInput")
    WpT = nc.dram_tensor("WpT", [128, KD, D], BF16, kind="ExternalInput")
    bp = nc.dram_tensor("bp", [1, D], BF16, kind="ExternalInput")
    # causal mask for the diagonal 128-block, replicated for the 2 heads
    mask2 = nc.dram_tensor("mask2", [128, 2, 128], F8E4, kind="ExternalInput")
    mask2b = nc.dram_tensor("mask2b", [128, 2, 128], BF16, kind="ExternalInput")
    idin = nc.dram_tensor("idin", [128, 128], BF16, kind="ExternalInput")
    sel = nc.dram_tensor("sel", [40, KD, 128], F32, kind="ExternalInput")
    # y rows: [0:HSL] = batch0 s-slice, [HSL:2*HSL] = batch1 s-slice
    y = nc.dram_tensor("y", [B * HSL, D], F32, kind="ExternalOutput")

    with tile.TileContext(nc) as tc:
        ctx_pools = [
            tc.tile_pool(name="persist", bufs=1),
            tc.tile_pool(name="dram", bufs=1, space="DRAM"),
            tc.tile_pool(name="wq", bufs=1),
            tc.tile_pool(name="xp", bufs=2),
            tc.tile_pool(name="qk", bufs=2),
            tc.tile_pool(name="vp", bufs=2),
            tc.tile_pool(name="at", bufs=4),
            tc.tile_pool(name="at16", bufs=2),
            tc.tile_pool(name="xsl", bufs=2),
            tc.tile_pool(name="prj", bufs=2),
            tc.tile_pool(name="yo", bufs=2),
            tc.tile_pool(name="ps_mix", bufs=2, space="PSUM"),
            tc.tile_pool(name="ps_sc", bufs=2, space="PSUM"),
            tc.tile_pool(name="ps_oT", bufs=2, space="PSUM"),
        ]
        import contextlib

        with contextlib.ExitStack() as stk:
            (
                persist, dram, wpool, xpool, qkpool, vppool, atpool,
                at16pool, xslpool,
                prjpool, ypool, ps_mix, ps_sc, ps_oT,
            ) = [stk.enter_context(p) for p in ctx_pools]

            # ---- critical-path first: small constants, weights, batch-0 x ----
            ident = persist.tile([128, 128], BF16)
            nc.sync.dma_start(out=ident, in_=idin[:, :])
            mask_sb = persist.tile([128, 2, 128], F8E4)
            nc.sync.dma_start(out=mask_sb, in_=mask2[:, :, :])
            mask_sb16 = persist.tile([128, 2, 128], BF16)
            nc.sync.dma_start(out=mask_sb16, in_=mask2b[:, :, :])
            wqk8_sb = wpool.tile([128, 2, KT, 2, 128], F8E4, tag="wqk8")
            nc.sync.dma_start(out=wqk8_sb, in_=Wqk8[:, :, :, :, :])
            wv_sb = wpool.tile([128, KD, 128], BF16, tag="wv")
            nc.sync.dma_start(out=wv_sb, in_=Wv[:, :, :])

            _warm_n = [0]

            def warm(k):
                # back-to-back thin matmuls: keeps the PE HAM activity window
                # busy across DMA / collective waits; 1-col stationary makes
                # the LDWEIGHTS cost negligible
                _warm_n[0] += 1
                wps = ps_mix.tile(
                    [128, SQ], F32, tag="mix", name=f"warm_{_warm_n[0]}"
                )
                for _ in range(k):
                    nc.tensor.matmul(
                        wps[0:1, 0:128], ident[:, 0:1], ident, start=True, stop=True
                    )

            warm(26)  # bridge until the first x slab lands

            x8_sb, x_sb = {}, {}

            HS = S // 2  # half-sequence slab width (bigger DMA pieces)
            CPH = HS // SQ  # query chunks per half-slab

            def load_x8slab(b, hn):
                # fp8 x half-slab hn for batch b (1KB contiguous pieces)
                if b not in x8_sb:
                    x8_sb[b] = [
                        xpool.tile(
                            [128, 2, S], F8E4, tag=f"x8{tt}", name=f"x8_{b}_{tt}"
                        )
                        for tt in range(KT)
                    ]
                # scalar HWDGE queue: runs parallel to the sync queue that
                # carries the (2x bigger) bf16 stream
                for tt in range(KT):
                    nc.scalar.dma_start(
                        out=x8_sb[b][tt][:, :, HS * hn : HS * (hn + 1)],
                        in_=xT8[b, tt, :, :, HS * hn : HS * (hn + 1)],
                    )

            def load_xslab(b, hn):
                # rolling bf16 x half-slab for the v projection (2KB pieces)
                xs = xslpool.tile(
                    [128, KD, HS], BF16, tag="xsl", name=f"x_{b}_{hn}"
                )
                for t in range(KD):
                    nc.sync.dma_start(
                        out=xs[:, t, :],
                        in_=xT[b, 128 * t : 128 * (t + 1), HS * hn : HS * (hn + 1)],
                    )
                x_sb[(b, hn)] = xs

            load_x8slab(0, 0)
            load_xslab(0, 0)

            ones_sb = persist.tile([1, 128], BF16)
            nc.vector.memset(ones_sb, 1.0)


            a2a_in = {
                b: dram.tile([NCORES, 65, 2, HSL], BF16, name=f"a2a_in_{b}")
                for b in range(B)
            }
            a2a_out = {
                b: dram.tile([NCORES, 65, 2, HSL], BF16, name=f"a2a_out_{b}")
                for b in range(B)
            }
            qkvT = {}
            vp8 = {}

            def _qkv_dst(b):
                if b not in qkvT:
                    qkvT[b] = qkpool.tile(
                        [128, 3, S], BF16, tag="qkvT", name=f"qkvT_{b}"
                    )
                return qkvT[b]

            prog_qk = set()
            prog_vu = {0: 0, 1: 0}

            def emit_qk_group(b, w, n):
                # fp8 DoubleRow: contraction pairs packed, 2x PE throughput
                dst = _qkv_dst(b)[:, w, SQ * n : SQ * (n + 1)]
                ps = ps_mix.tile([128, SQ], F32, tag="mix", name=f"psqk_{b}_{w}_{n}")
                for tt in range(KT):
                    nc.tensor.matmul(
                        ps,
                        wqk8_sb[:, w, tt, :, :],
                        x8_sb[b][tt][:, :, SQ * n : SQ * (n + 1)],
                        start=(tt == 0),
                        stop=(tt == KT - 1),
                        perf_mode=DoubleRow,
                    )
                nc.vector.tensor_copy(dst, ps)
                if w == 1:
                    prog_qk.add((b, n))

            def emit_v_group(b, n):
                # v projection in bf16 (values need the precision)
                dst = _qkv_dst(b)[:, 2, SQ * n : SQ * (n + 1)]
                ps = ps_mix.tile([128, SQ], F32, tag="mix", name=f"psv_{b}_{n}")
                xs = x_sb[(b, n // CPH)]
                c0 = (n % CPH) * SQ
                for t in range(KD):
                    nc.tensor.matmul(
                        ps,
                        wv_sb[:, t, :],
                        xs[:, t, c0 : c0 + SQ],
                        start=(t == 0),
                        stop=(t == KD - 1),
                    )
                nc.vector.tensor_copy(dst, ps)

            vp0 = {}

            def emit_v_unit(b, i):
                # transpose key-tile i of v into the fp8 DoubleRow pair layout
                # vp8[p, pair, kt, hs, m] (m stride padded to 72 for step%16);
                # key tiles 0-3 also get a bf16 copy for the chunk-0 AV path
                if b not in vp8:
                    vp8[b] = vppool.tile(
                        [128, NT // 2, 2, 2, 72], F8E4, tag="vp", name=f"vp_{b}"
                    )
                    nc.vector.memset(vp8[b][:, :, :, :, 64], 1.0)
                    vp0[b] = vppool.tile(
                        [128, 4, 2, 65], BF16, tag="vp0", name=f"vp0_{b}"
                    )
                    nc.vector.memset(vp0[b][:, :, :, 64], 1.0)
                pst = ps_mix.tile([128, 128], BF16, tag="mix", name=f"psvt_{b}_{i}")
                nc.tensor.transpose(
                    pst, qkvT[b][:, 2, 128 * i : 128 * (i + 1)], ident[:, :]
                )
                for hs in range(2):
                    nc.vector.tensor_copy(
                        vp8[b][:, i // 2, i % 2, hs, 0:64],
                        pst[:, 64 * hs : 64 * hs + 64],
                    )
                    if i < 4:
                        nc.vector.tensor_copy(
                            vp0[b][:, i, hs, 0:64],
                            pst[:, 64 * hs : 64 * hs + 64],
                        )
                prog_vu[b] += 1

            # ---- attention tick machinery ----
            att_state = {}

            def attn_tick(b, n, j):
                """Scores for key-tile j of chunk (b, n), both heads packed as
                2-way row-tiled K=64 matmuls, then one exp over both heads."""
                st = att_state.setdefault((b, n), {"at": {}, "ot": {}})
                p = j // 2
                off = max(0, 128 * j - SQ * n)
                fp8 = n > 0  # chunk 0 runs bf16 AV (few keys: fp8 at too lossy)
                if j % 2 == 0:
                    if fp8:
                        at8 = atpool.tile(
                            [128, 2, 2, SQ], F8E4, tag="at", name=f"at_{b}_{n}_{p}"
                        )
                        off_odd = max(0, 128 * (j + 1) - SQ * n)
                        if off_odd > off:
                            # odd key-tile's dead zone: out-of-causal, zero it
                            nc.vector.memset(at8[:, 1, :, off:off_odd], 0.0)
                    else:
                        at8 = at16pool.tile(
                            [128, 2, 2, SQ], BF16, tag="at16", name=f"at_{b}_{n}_{p}"
                        )
                    st["at"][p] = at8
                else:
                    at8 = st["at"][p]
                sc = ps_sc.tile([128, 2, SQ], F32, tag="sc", name=f"sc_{b}_{n}_{j}")
                for h in range(2):
                    nc.tensor.matmul(
                        sc[:, h, off:],
                        qkvT[b][64 * h : 64 * h + 64, 1, 128 * j : 128 * (j + 1)],
                        qkvT[b][64 * h : 64 * h + 64, 0, SQ * n + off : SQ * (n + 1)],
                        start=True,
                        stop=True,
                    )
                nc.scalar.activation(
                    at8[:, j % 2, :, off:],
                    sc[:, :, off:],
                    Exp,
                    scale=SCALE / (QK_PRESCALE * QK_PRESCALE),
                )
                if j >= 4 * n:
                    # diagonal block: zero the strict upper triangle
                    nc.vector.tensor_mul(
                        at8[:, j % 2, :, off : off + 128],
                        at8[:, j % 2, :, off : off + 128],
                        mask_sb if fp8 else mask_sb16,
                    )

            def av_pair(b, n, p, jmax):
                """AV for key-tile pair p of chunk (b, n): one fp8 DoubleRow
                matmul per head consuming both key tiles; den rides as row 64."""
                st = att_state[(b, n)]
                at8 = st["at"][p]
                off = max(0, 128 * 2 * p - SQ * n)
                last = 2 * p + 2 >= jmax
                for h in range(2):
                    if h not in st["ot"]:
                        st["ot"][h] = ps_oT.tile(
                            [65, SQ], F32, tag="ot", name=f"ot_{b}_{n}_{h}"
                        )
                    if n > 0:
                        nc.tensor.matmul(
                            st["ot"][h][:, off:],
                            vp8[b][:, p, :, h, 0:65],
                            at8[:, :, h, off:],
                            start=(p == 0),
                            stop=last,
                            perf_mode=DoubleRow,
                        )
                    else:
                        for k in range(2):
                            j = 2 * p + k
                            offk = max(0, 128 * j - SQ * n)
                            nc.tensor.matmul(
                                st["ot"][h][:, offk:],
                                vp0[b][:, j, h, 0:65],
                                at8[:, k, h, offk:],
                                start=(j == 0),
                                stop=(j == jmax - 1),
                            )
                if last:
                    # [65, dest, hs, HSL]: each dest's payload is contiguous
                    # per partition row (1KB pieces for the staging DMA)
                    DPC = SQ // HSL
                    oT = prjpool.tile(
                        [65, DPC, 2, HSL], BF16, tag="oT", name=f"oT_{b}_{n}"
                    )
                    for h in range(2):
                        nc.vector.tensor_copy(
                            oT[:, :, h, :],
                            st["ot"][h][:, :].rearrange("p (d s) -> p d s", d=DPC),
                        )
                    # staging for the dest cores this chunk completes
                    for d in range(n * DPC, (n + 1) * DPC):
                        nc.sync.dma_start(
                            out=a2a_in[b][d, :, :, :],
                            in_=oT[:, d - n * DPC, :, :],
                        )
                    if n == NCH - 1:
                        nc.gpsimd.collective_compute(
                            "AllToAll",
                            mybir.AluOpType.bypass,
                            replica_groups=[list(range(NCORES))],
                            ins=[a2a_in[b][:, :, :, :].opt()],
                            outs=[a2a_out[b][:, :, :, :].opt()],
                        )

            # ---- phase D (output projection) pieces for batch b ----
            dstate = {}

            def emit_D_head(b):
                st_ = dstate.setdefault(b, {})
                den = prjpool.tile([40, HSL], BF16, tag="den", name=f"den_{b}")
                den32 = prjpool.tile([40, HSL], F32, tag="den32", name=f"den32_{b}")
                rcp32 = prjpool.tile([40, HSL], F32, tag="rcp32", name=f"rcp32_{b}")
                rcp = prjpool.tile([40, HSL], F32R, tag="rcp", name=f"rcp_{b}")
                st_["rcp"] = rcp
                st_["onrm"] = prjpool.tile(
                    [128, KD, HSL], BF16, tag="onrm", name=f"onrm_{b}"
                )
                nc.vector.memset(den32, 1.0)
                for hs in range(2):
                    r0 = 32 * hs
                    nc.sync.dma_start(
                        out=den[r0 : r0 + KD, :], in_=a2a_out[b][:, 64, hs, :]
                    )
                    nc.vector.tensor_copy(
                        den32[r0 : r0 + KD, :], den[r0 : r0 + KD, :]
                    )
                with nc.allow_low_precision(reason="softmax denom recip"):
                    nc.vector.reciprocal_approx_fast(rcp32, den32)
                nc.vector.tensor_copy(rcp, rcp32)

            def emit_D_norm(b, t):
                st_ = dstate[b]
                for hs in range(2):
                    nc.sync.dma_start(
                        out=st_["onrm"][64 * hs : 64 * hs + 64, t, :],
                        in_=a2a_out[b][t, 0:64, hs, :],
                    )
                bc = ps_mix.tile([128, HSL], F32, tag="mix", name=f"bc_{b}_{t}")
                nc.tensor.matmul(
                    bc, sel_sb[:, t, :], st_["rcp"], start=True, stop=True
                )
                nc.vector.tensor_mul(
                    st_["onrm"][:, t, :], st_["onrm"][:, t, :], bc
                )

            def emit_D_group(b, st, nn, tail=False):
                st_ = dstate[b]
                acc = ps_mix.tile(
                    [128, 512], F32, tag="mix", name=f"acc_{b}_{st}_{nn}"
                )
                for t in range(KD):
                    nc.tensor.matmul(
                        acc,
                        st_["onrm"][:, t, 128 * st : 128 * (st + 1)],
                        wpT_sb[:, t, 512 * nn : 512 * (nn + 1)],
                        start=(t == 0),
                        stop=False,
                    )
                nc.tensor.matmul(
                    acc, ones_sb, bp_sb[:, 512 * nn : 512 * (nn + 1)],
                    start=False, stop=True,
                )
                yt = ypool.tile([128, 512], F32, tag="y", name=f"y_{b}_{st}_{nn}")
                if tail:
                    nc.scalar.copy(yt, acc)
                else:
                    nc.vector.tensor_copy(yt, acc)
                nc.sync.dma_start(
                    out=y[
                        b * HSL + 128 * st : b * HSL + 128 * (st + 1),
                        512 * nn : 512 * (nn + 1),
                    ],
                    in_=yt,
                )

            # ---- the unified pipeline ----
            # lead-in: minimum to start (b0, chunk0): qk+v group 0, v-units 0-3
            emit_qk_group(0, 0, 0)
            emit_qk_group(0, 1, 0)
            emit_v_group(0, 0)
            for i in range(4):
                emit_v_unit(0, i)

            # tick index bookkeeping
            ticks_b0 = sum(4 * n + 4 for n in range(NCH))
            total_ticks = 2 * ticks_b0

            def _wp_loads():
                nc.sync.dma_start(out=wpT_sb, in_=WpT[:, :, :])
                nc.sync.dma_start(out=bp_sb, in_=bp[:, :])
                nc.sync.dma_start(out=sel_sb, in_=sel[:, :, :].bitcast(F32R))

            # chunk start ticks for the interleaved sequence (NCH==4):
            #   b0: c0@0 c1@4 c2@12 c3@28   b1: c0@24 c1@44 c2@52 c3@64
            # filler list: (gate_tick, closure), consumed strictly in order;
            # each unit is gated shortly before its consumer chunk starts so
            # the per-tick PE load stays level across the whole pipeline
            fillers = []

            def proj_units(b, n, gq, gv):
                fillers.append((gq, lambda: emit_qk_group(b, 0, n)))
                fillers.append((gq, lambda: emit_qk_group(b, 1, n)))
                fillers.append((gv, lambda: emit_v_group(b, n)))
                for i in range(4 * n, 4 * n + 4):
                    fillers.append((gv, lambda i=i: emit_v_unit(b, i)))

            if NCH == 4:
                proj_units(0, 1, 0, 0)
                fillers.append((0, lambda: load_x8slab(0, 1)))
                fillers.append((0, lambda: load_xslab(0, 1)))
                fillers.append((2, lambda: load_x8slab(1, 0)))
                fillers.append((2, lambda: load_xslab(1, 0)))
                fillers.append((3, _wp_loads))
                proj_units(0, 2, 5, 6)
                fillers.append((8, lambda: load_x8slab(1, 1)))
                fillers.append((8, lambda: load_xslab(1, 1)))
                proj_units(1, 0, 14, 16)
                proj_units(0, 3, 18, 20)
                proj_units(1, 1, 36, 38)
                proj_units(1, 2, 44, 46)
                proj_units(1, 3, 56, 58)
            else:
                for n in range(1, NCH):
                    if n == 1 and NCH > CPH:
                        fillers.append((0, lambda: load_x8slab(0, 1)))
                        fillers.append((0, lambda: load_xslab(0, 1)))
                    proj_units(0, n, 0, 0)
                fillers.append((2, lambda: load_x8slab(1, 0)))
                fillers.append((2, lambda: load_xslab(1, 0)))
                fillers.append((3, _wp_loads))
                for n in range(NCH):
                    g = 4 if n == 0 else max(4, ticks_b0 + 4 * n - 6)
                    if n == 1 and NCH > CPH:
                        fillers.append((g, lambda: load_x8slab(1, 1)))
                        fillers.append((g, lambda: load_xslab(1, 1)))
                    proj_units(1, n, g, g + 1)



            wpT_sb = persist.tile([128, KD, D], BF16)
            bp_sb = persist.tile([1, D], BF16)
            sel_sb = persist.tile([40, KD, 128], F32R)

            # pending AV pairs: issue each one tick after its exp completes
            pend = []
            tick_no = [0]

            def drain_until(cond):
                # force-emit fillers (in order) until cond() holds
                while not cond():
                    assert fi_[0] < len(fillers), "filler list exhausted"
                    fillers[fi_[0]][1]()
                    fi_[0] += 1

            def run_tick(b, n, j, jmax):
                i = tick_no[0]
                drain_until(lambda: (b, n) in prog_qk)
                attn_tick(b, n, j)
                # pace fillers
                _credit[0] = min(
                    _credit[0] + (len(fillers) - fi_[0]) / max(1, total_ticks - i),
                    3.0,
                )
                popped = 0
                while (
                    _credit[0] >= 1.0
                    and fi_[0] < len(fillers)
                    and fillers[fi_[0]][0] <= i
                ):
                    fillers[fi_[0]][1]()
                    fi_[0] += 1
                    popped += 1
                    _credit[0] -= 1.0
                if popped == 0:
                    warm(2)
                # consume one pending AV pair (stagger >= 1 tick after its exp)
                while pend and pend[0][3] <= i - 1:
                    bb, nn_, pp, _, jm = pend.pop(0)
                    drain_until(lambda: prog_vu[bb] >= 2 * pp + 2)
                    av_pair(bb, nn_, pp, jm)
                if j % 2 == 1:
                    pend.append((b, n, j // 2, i, jmax))
                tick_no[0] += 1

            _credit = [0.0]
            fi_ = [0]

            # batch-interleaved chunk order: batch 0 finishes (and fires its
            # collective) well before batch 1, so the two AllToAlls never
            # queue up behind each other on the collective cores
            if NCH == 4:
                seq_c = [(0, 0), (0, 1), (0, 2), (1, 0), (0, 3), (1, 1), (1, 2), (1, 3)]
            else:
                seq_c = [(0, n) for n in range(NCH)] + [(1, n) for n in range(NCH)]
            for b, n in seq_c:
                jmax = 4 * n + 4
                for j in range(jmax):
                    run_tick(b, n, j, jmax)

            # flush remaining AV pairs (the last chunk's tail) — this emits
            # b1's staging + collective trigger; nothing a2a-dependent may
            # precede it in the PE queue
            while pend:
                bb, nn_, pp, _, jm = pend.pop(0)
                drain_until(lambda: prog_vu[bb] >= 2 * pp + 2)
                av_pair(bb, nn_, pp, jm)
                warm(2)
            # any unconsumed fillers
            while fi_[0] < len(fillers):
                fillers[fi_[0]][1]()
                fi_[0] += 1

            # phase D for batch 0 (its collective fired at the b0/b1 tick
            # boundary and has long landed); then batch 1 behind its own
            # collective, with a warm bridge across the wait
            warm(60)
            emit_D_head(0)
            for t in range(KD):
                emit_D_norm(0, t)
                warm(3)
            for st in range(NST):
                for nn in range(NNN):
                    emit_D_group(0, st, nn)
                    warm(4)
            warm(120)
            emit_D_head(1)
            for t in range(KD):
                emit_D_norm(1, t)
                warm(3)
            for st in range(NST):
                for nn in range(NNN):
                    emit_D_group(1, st, nn, tail=True)
                    warm(4)

    nc.compile()
    return nc


_built = {}


def get_nc(S=2048):
    if S not in _built:
        _built[S] = build(S)
    return _built[S]


def prep_inputs(x, Wq, Wk, Wv, Wp, bp):
    """Host-side shard prep. Returns per-core input maps."""
    import ml_dtypes

    BF = ml_dtypes.bfloat16
    F8 = ml_dtypes.float8_e4m3fn
    x = np.ascontiguousarray(np.asarray(x, dtype=np.float32))
    Wq, Wk, Wv = (np.asarray(w, dtype=np.float32) for w in (Wq, Wk, Wv))
    Wp = np.asarray(Wp, dtype=np.float32)
    bp = np.asarray(bp, dtype=np.float32)
    xT32 = np.ascontiguousarray(x.transpose(0, 2, 1))
    xT = xT32.astype(BF)
    KD = D // 128
    KT = KD // 2
    S = x.shape[1]
    # fp8 x for DoubleRow projections: [b, tt, p, kt, s]
    xT8 = np.ascontiguousarray(
        xT32.reshape(x.shape[0], KT, 2, 128, S).transpose(0, 1, 3, 2, 4)
    ).astype(F8)
    # WpT pre-arranged for SBUF: [p, t, i] with row t*128+p of Wp.T
    WpT = np.ascontiguousarray(
        Wp.T.reshape(KD, 128, D).transpose(1, 0, 2)
    ).astype(BF)
    mask1 = np.triu(np.ones((128, 128), dtype=np.float32))
    mask2_32 = np.ascontiguousarray(np.stack([mask1, mask1], axis=1))
    mask2 = mask2_32.astype(F8)
    mask2b = mask2_32.astype(BF)
    idin = np.eye(128, dtype=np.float32).astype(BF)
    sel = np.zeros((40, KD, 128), dtype=np.float32)
    for t in range(KD):
        sel[t, t, 0:64] = 1.0           # head 2t     -> den row t
        sel[32 + t, t, 64:128] = 1.0    # head 2t + 1 -> den row 32 + t
    in_maps = []
    QKS = QK_PRESCALE
    for c in range(NCORES):
        h0 = 2 * c
        wqk = np.stack(
            [
                np.concatenate([Wq[h0], Wq[h0 + 1]], axis=1),
                np.concatenate([Wk[h0], Wk[h0 + 1]], axis=1),
            ]
        ) * QKS  # [2, D, 128]
        # DoubleRow layout: [p, w, tt, kt, m]
        wqk8 = np.ascontiguousarray(
            wqk.reshape(2, KT, 2, 128, 128).transpose(3, 0, 1, 2, 4)
        ).astype(F8)
        wv = np.concatenate([Wv[h0], Wv[h0 + 1]], axis=1)  # [D, 128]
        wv = np.ascontiguousarray(
            wv.reshape(KD, 128, 128).transpose(1, 0, 2)
        ).astype(BF)  # [p, t, m]
        in_maps.append(
            {
                "xT8": xT8,
                "xT": xT,
                "Wqk8": wqk8,
                "Wv": wv,
                "WpT": WpT,
                "bp": bp.reshape(1, D).astype(BF),
                "mask2": mask2,
                "mask2b": mask2b,
                "idin": idin,
                "sel": sel,
            }
        )
    return in_maps


# inputs identical across cores are passed replicated (shipped once, not 8x)
_REPLICATED = {"xT8", "xT", "WpT", "bp", "mask2", "mask2b", "idin", "sel"}

_runners = {}


def _get_runner(S):
    """Cached jitted SPMD callable for the built module."""
    if S in _runners:
        return _runners[S]
    import jax
    import concourse.mybir as _mybir
    from concourse import bass2jax
    from jax.experimental.shard_map import shard_map
    from jax.sharding import Mesh, PartitionSpec

    nc = get_nc(S)
    bass2jax.install_neuronx_cc_hook()

    in_names, out_names, out_avals = [], [], []
    partition_name = nc.partition_id_tensor.name if nc.partition_id_tensor else None
    for alloc in nc.m.functions[0].allocations:
        if not isinstance(alloc, _mybir.MemoryLocationSet):
            continue
        name = alloc.memorylocations[0].name
        if alloc.kind == "ExternalInput":
            if name != partition_name:
                in_names.append(name)
        elif alloc.kind == "ExternalOutput":
            out_names.append(name)
            out_avals.append(
                jax.core.ShapedArray(tuple(alloc.tensor_shape), _mybir.dt.np(alloc.dtype))
            )
    n_params = len(in_names)
    all_in_names = list(in_names) + list(out_names)
    if partition_name is not None:
        all_in_names.append(partition_name)

    def _body(*args):
        operands = list(args)
        if partition_name is not None:
            operands.append(bass2jax.partition_id_tensor())
        outs = bass2jax._bass_exec_p.bind(
            *operands,
            out_avals=tuple(out_avals),
            in_names=tuple(all_in_names),
            out_names=tuple(out_names),
            lowering_input_output_aliases=(),
            sim_require_finite=True,
            sim_require_nnan=True,
            nc=nc,
        )
        return tuple(outs)

    devices = jax.devices()[:NCORES]
    mesh = Mesh(np.asarray(devices), ("core",))
    in_specs = tuple(
        PartitionSpec() if nm in _REPLICATED else PartitionSpec("core")
        for nm in in_names
    ) + (PartitionSpec("core"),) * len(out_names)
    out_specs = (PartitionSpec("core"),) * len(out_names)
    donate = tuple(range(n_params, n_params + len(out_names)))
    fn = jax.jit(
        shard_map(_body, mesh=mesh, in_specs=in_specs, out_specs=out_specs, check_rep=False),
        donate_argnums=donate,
        keep_unused=True,
    )
    r = (fn, in_names, out_names, out_avals, mesh)
    _runners[S] = r
    return r


class _Res:
    def __init__(self, results):
        self.results = results
        self.exec_time_ns = None


def run(x, Wq, Wk, Wv, Wp, bp, timings=None):
    import time as _time

    S = x.shape[1]
    t0 = _time.perf_counter()
    fn, in_names, out_names, out_avals, mesh = _get_runner(S)
    t1 = _time.perf_counter()
    in_maps = prep_inputs(x, Wq, Wk, Wv, Wp, bp)
    t2 = _time.perf_counter()
    args = []
    for nm in in_names:
        if nm in _REPLICATED:
            args.append(in_maps[0][nm])
        else:
            args.append(np.concatenate([in_maps[c][nm] for c in range(NCORES)], axis=0))
    zero_outs = [
        np.zeros((NCORES * av.shape[0], *av.shape[1:]), av.dtype) for av in out_avals
    ]
    t3 = _time.perf_counter()
    out_arrs = fn(*args, *zero_outs)
    out_np = [np.asarray(o) for o in out_arrs]
    t4 = _time.perf_counter()
    results = [
        {
            nm: out_np[i].reshape(NCORES, *out_avals[i].shape)[c]
            for i, nm in enumerate(out_names)
        }
        for c in range(NCORES)
    ]
    if timings is not None:
        timings.update(
            runner=t1 - t0, prep=t2 - t1, concat=t3 - t2, exec=t4 - t3
        )
    return _assemble_y([results[c]["y"] for c in range(NCORES)]), _Res(results)


def _assemble_y(per_core):
    """per-core y is [B*HSL, D]: rows [b*HSL:(b+1)*HSL] = batch b, s-slice c."""
    HSL = per_core[0].shape[0] // B
    S = HSL * NCORES
    out = np.empty((B, S, D), dtype=per_core[0].dtype)
    for c in range(NCORES):
        for b in range(B):
            out[b, HSL * c : HSL * (c + 1), :] = per_core[c][b * HSL : (b + 1) * HSL]
    return out


def kernel(x, Wq, Wk, Wv, Wp, bp):
    out, _ = run(x, Wq, Wk, Wv, Wp, bp)
    return out


# ---------------------------------------------------------------------------
# NTFF profiling support (test harness only; not needed for kernel()).
# ---------------------------------------------------------------------------

def _ntff_hook():
    import contextlib
    import ctypes

    lib = ctypes.CDLL("/opt/axon/libaxon_pjrt.so")
    lib.axon_start_nrt_profile.argtypes = [
        ctypes.POINTER(ctypes.c_int64),
        ctypes.c_size_t,
    ]
    lib.axon_start_nrt_profile.restype = ctypes.c_int64
    lib.axon_stop_nrt_profile.argtypes = [ctypes.c_char_p]
    lib.axon_stop_nrt_profile.restype = ctypes.c_int64

    @contextlib.contextmanager
    def _hook(output_dir, device_ids):
        import jax

        jax.devices()
        if device_ids:
            ids = (ctypes.c_int64 * len(device_ids))(*device_ids)
            rc = lib.axon_start_nrt_profile(ids, len(device_ids))
        else:
            rc = lib.axon_start_nrt_profile(None, 0)
        if rc != 0:
            raise RuntimeError(f"axon_start_nrt_profile rc={rc}")
        try:
            yield
        finally:
            n = lib.axon_stop_nrt_profile(str(output_dir).encode())
            print(f"profile: {n} file(s) written to {output_dir}")

    return _hook


def run_traced(x, Wq, Wk, Wv, Wp, bp, outdir=None, cores=(0,)):
    """Run once under NTFF profiling; returns (out, exec_time_ns, trace_path)."""
    import glob
    import tempfile

    import gauge.profiler
    from concourse._compat import FishPath

    S = x.shape[1]
    fn, in_names, out_names, out_avals, mesh = _get_runner(S)
    in_maps = prep_inputs(x, Wq, Wk, Wv, Wp, bp)
    args = []
    for nm in in_names:
        if nm in _REPLICATED:
            args.append(in_maps[0][nm])
        else:
            args.append(np.concatenate([in_maps[c][nm] for c in range(NCORES)], axis=0))
    zero_outs = [
        np.zeros((NCORES * av.shape[0], *av.shape[1:]), av.dtype) for av in out_avals
    ]
    # warm (compile + first exec)
    out_arrs = fn(*args, *zero_outs)
    _ = [np.asarray(o) for o in out_arrs]

    if outdir is None:
        outdir = tempfile.mkdtemp(prefix="ntff_")
    hook = _ntff_hook()
    zero_outs = [
        np.zeros((NCORES * av.shape[0], *av.shape[1:]), av.dtype) for av in out_avals
    ]
    with hook(outdir, list(cores)):
        out_arrs = fn(*args, *zero_outs)
        out_np = [np.asarray(o) for o in out_arrs]

    ntffs = glob.glob(f"{outdir}/*.ntff")
    if not ntffs:
        print(f"no NTFF files in {outdir}")
        return None, None, None
    nc = get_nc(S)
    profile = gauge.profiler.Profile(
        profile_path=FishPath(outdir),
        kernel_dev_mode=True,
        profile_on_exit=False,
        bass_kernel=nc.m,
        offline_processing=True,
        fname="*_body*",
        metadata={"artifacts_path": outdir},
    )
    results = profile.to_perfetto(model_index=tuple(range(len(cores))))
    exec_ns = max(r.exec_time_ns for r in results)
    yfull = _assemble_y(
        [out_np[out_names.index("y")].reshape(NCORES, -1, D)[c] for c in range(NCORES)]
    )
    return yfull, exec_ns, results[0].trace_path
